# revision 1
# baseline (speedup 1.0000x reference)
"""Trainium2 Bass kernel for a pre-LN transformer block (B=128,T=256,C=384,H=6,D=64).

Data-parallel over batch across 8 NeuronCores (16 batches/core), processed in
pairs so the QKV and FFN1 matmuls stream a 512-wide moving operand (two
batches' tokens side by side). All matmuls run as float32r. LN gamma/beta are
folded into the weights on the host; device LN is (x - mu) * rstd via
bn_stats/bn_aggr. Attention uses the transposed-score orientation ([S,T]):
softmax denominators come from an all-ones matmul that also broadcasts them
across partitions, causal masking zeroes exp(scores) with
gpsimd.affine_select, and normalization happens during the attn@v PSUM
evacuation.
"""

import sys

if "/opt/trn_rl_repo" not in sys.path:
    sys.path.insert(0, "/opt/trn_rl_repo")

import numpy as np

import concourse.bass as bass
import concourse.mybir as mybir
import concourse.tile as tile
from concourse import bacc

# All ACT functions used here (Exp, Ln, Relu, Identity, Copy) live in the
# 'natural_log_exp_and_others' table set. Blank the other sets (preserving
# dict order, which defines act_func_set_id) so the table-load fixpoint
# settles on a single ACT_TABLE_LOAD instead of thrashing sets per batch.
_KEEP_ACT_SET = "natural_log_exp_and_others"
_orig_get_act_tables = bacc.get_activation_tables


def _one_set_tables(arch):
    t = _orig_get_act_tables(arch)
    assert _KEEP_ACT_SET in t
    return {k: (v if k == _KEEP_ACT_SET else set()) for k, v in t.items()}


bacc.get_activation_tables = _one_set_tables

F32 = mybir.dt.float32
F32R = mybir.dt.float32r
AF = mybir.ActivationFunctionType
ALU = mybir.AluOpType

B, T, C, H, D = 128, 256, 384, 6, 64
NCORES = 8
BL = B // NCORES          # batches per core
F = 4 * C                 # 1536
P = 128
TCH = T // P              # 2 token chunks
CCH = C // P              # 3 channel chunks
FCH = F // P              # 12 ffn chunks
HD = H * D                # 384
SCALE = float(C) ** -0.5  # reference scales by full model dim
EPS = 1e-5


def build_program(bl=BL, flags=frozenset(), repeat=1,
                  tr_split=False, tr_bufs=1, ps1_bufs=6, wk_bufs=2):
    """Per-core Bass program. `flags` lists nonzero bias terms
    ('qb','kb','vb','bo','b1','b2'). `repeat` wraps the whole computation in
    a hardware loop (benchmarking only)."""
    assert bl % 2 == 0
    use_qb = "qb" in flags
    use_kb = "kb" in flags
    use_vb = "vb" in flags
    use_bo = "bo" in flags
    use_b1 = "b1" in flags
    use_b2 = "b2" in flags

    nc = bacc.Bacc("TRN2", target_bir_lowering=False, debug=False,
                   num_devices=NCORES)

    x_d = nc.dram_tensor("x", [bl, T, C], F32, kind="ExternalInput")
    wq_d = nc.dram_tensor("wq", [P, CCH, HD], F32R, kind="ExternalInput")
    wk_d = nc.dram_tensor("wk", [P, CCH, HD], F32R, kind="ExternalInput")
    wv_d = nc.dram_tensor("wv", [P, CCH, HD], F32R, kind="ExternalInput")
    qb_d = nc.dram_tensor("qb", [P, CCH], F32, kind="ExternalInput")
    kb_d = nc.dram_tensor("kb", [P, CCH], F32, kind="ExternalInput")
    vb_d = nc.dram_tensor("vb", [1, HD], F32R, kind="ExternalInput")
    wo_d = nc.dram_tensor("wo", [D, H, C], F32R, kind="ExternalInput")
    bo_d = nc.dram_tensor("bo", [1, C], F32R, kind="ExternalInput")
    w1_d = nc.dram_tensor("w1", [P, CCH, F], F32R, kind="ExternalInput")
    b1_d = nc.dram_tensor("b1c", [P, FCH], F32, kind="ExternalInput")
    w2_d = nc.dram_tensor("w2", [P, FCH, C], F32R, kind="ExternalInput")
    b2_d = nc.dram_tensor("b2", [1, C], F32R, kind="ExternalInput")
    id_d = nc.dram_tensor("ident", [P, P], F32R, kind="ExternalInput")
    on_d = nc.dram_tensor("onesm", [P, P], F32R, kind="ExternalInput")
    tl_d = nc.dram_tensor("trilm", [P, P], F32R, kind="ExternalInput")
    ng_d = nc.dram_tensor("negm", [P, TCH, T], F32R, kind="ExternalInput")
    y_d = nc.dram_tensor("y", [bl, T, C], F32, kind="ExternalOutput")

    with tile.TileContext(nc) as tc:
        import contextlib
        with (
            tc.tile_pool(name="wpool", bufs=1) as wp,
            tc.tile_pool(name="work", bufs=wk_bufs) as wk_pool,
            tc.tile_pool(name="big", bufs=1) as bigp,
            (contextlib.nullcontext(None) if tr_split else
             tc.tile_pool(name="ps_tr", bufs=tr_bufs, space="PSUM")) as pstr,
            tc.tile_pool(name="ps_one", bufs=ps1_bufs, space="PSUM") as ps1,
        ):
            # ---- load weights/constants once ----
            wq = wp.tile([P, CCH, HD], F32R)
            wkk = wp.tile([P, CCH, HD], F32R)
            wv = wp.tile([P, CCH, HD], F32R)
            wo = wp.tile([D, H, C], F32R)
            w1 = wp.tile([P, CCH, F], F32R)
            w2 = wp.tile([P, FCH, C], F32R)
            ident = wp.tile([P, P], F32R)
            ones_t = wp.tile([P, P], F32R)
            trilm = wp.tile([P, P], F32R)
            negm = wp.tile([P, TCH, T], F32R)
            epsb = wp.tile([P, 1], F32)
            nc.gpsimd.memset(epsb[:], EPS)
            nc.sync.dma_start(wq[:], wq_d[:])
            nc.sync.dma_start(wkk[:], wk_d[:])
            nc.sync.dma_start(wv[:], wv_d[:])
            nc.sync.dma_start(wo[:], wo_d[:])
            nc.sync.dma_start(w1[:], w1_d[:])
            nc.sync.dma_start(w2[:], w2_d[:])
            nc.sync.dma_start(ident[:], id_d[:])
            nc.sync.dma_start(ones_t[:], on_d[:])
            nc.sync.dma_start(trilm[:], tl_d[:])
            nc.sync.dma_start(negm[:], ng_d[:])
            qb = kb = vb = bo = b1c = b2 = None
            if use_qb:
                qb = wp.tile([P, CCH], F32)
                nc.sync.dma_start(qb[:], qb_d[:])
            if use_kb:
                kb = wp.tile([P, CCH], F32)
                nc.sync.dma_start(kb[:], kb_d[:])
            if use_vb:
                vb = wp.tile([1, HD], F32R)
                nc.sync.dma_start(vb[:], vb_d[:])
            if use_bo:
                bo = wp.tile([1, C], F32R)
                nc.sync.dma_start(bo[:], bo_d[:])
            if use_b1:
                b1c = wp.tile([P, FCH], F32)
                nc.sync.dma_start(b1c[:], b1_d[:])
            if use_b2:
                b2 = wp.tile([1, C], F32R)
                nc.sync.dma_start(b2[:], b2_d[:])

            def layer_norm_T(src, dstT, i, evac_act):
                """src: [P, TCH, C] tokens-major tile. Writes (src-mu)*rstd
                transposed into dstT[:, :, i, :] ([P, CCH, 2, T] pair tile)."""
                st6 = wk_pool.tile([P, TCH, 6], F32, tag=f"st6_{i}")
                mv = wk_pool.tile([P, TCH, 2], F32, tag=f"mv_{i}")
                rstd = wk_pool.tile([P, TCH], F32, tag=f"rstd_{i}")
                for tch in range(TCH):
                    nc.vector.bn_stats(st6[:, tch, :], src[:, tch, :])
                    nc.vector.bn_aggr(mv[:, tch, :], st6[:, tch, :])
                # rstd = exp(-0.5 * ln(var + eps))
                nc.scalar.activation(rstd[:], mv[:, :, 1], AF.Ln, bias=epsb[:])
                nc.scalar.activation(rstd[:], rstd[:], AF.Exp, scale=-0.5)
                xn = wk_pool.tile([P, TCH, C], F32R, tag=f"xn_{i}", bufs=1)
                for tch in range(TCH):
                    nc.vector.tensor_scalar(
                        xn[:, tch, :], src[:, tch, :],
                        mv[:, tch, 0:1], rstd[:, tch:tch + 1],
                        ALU.subtract, ALU.mult,
                    )
                if tr_split:
                    trA = ps1.tile([P, 2, T], F32R, tag="ps1", name="trA")
                    trB = ps1.tile([P, T], F32R, tag="ps1", name="trB")

                    def _trdst(cc):
                        return trB if cc == 2 else trA[:, cc, :]
                else:
                    tr = pstr.tile([P, CCH, T], F32R, tag="tr")

                    def _trdst(cc):
                        return tr[:, cc, :]
                for tch in range(TCH):
                    for cc in range(CCH):
                        nc.tensor.transpose(
                            _trdst(cc)[:, tch * P:(tch + 1) * P],
                            xn[:, tch, cc * P:(cc + 1) * P],
                            ident[:],
                        )
                if tr_split:
                    if evac_act:
                        nc.scalar.copy(dstT[:, 0:2, i, :], trA[:])
                        nc.scalar.copy(dstT[:, 2, i, :], trB[:])
                    else:
                        nc.vector.tensor_copy(dstT[:, 0:2, i, :], trA[:])
                        nc.vector.tensor_copy(dstT[:, 2, i, :], trB[:])
                elif evac_act:
                    nc.scalar.copy(dstT[:, :, i, :], tr[:])
                else:
                    nc.vector.tensor_copy(dstT[:, :, i, :], tr[:])

            def body():
                for pb in range(bl // 2):
                    bp = (2 * pb, 2 * pb + 1)
                    xts = []
                    xnT2 = wk_pool.tile([P, CCH, 2, T], F32R, tag="xnT2")
                    for i, b in enumerate(bp):
                        xt = wk_pool.tile([P, TCH, C], F32, tag=f"xt{i}")
                        nc.sync.dma_start(
                            xt[:], x_d[b].rearrange("(tc p) c -> p tc c", p=P))
                        xts.append(xt)
                        layer_norm_T(xt, xnT2, i, evac_act=(i == 0))

                    # ---- q,k transposed [hd, (b,t)]; v natural [s, hd] ----
                    qsb2 = wk_pool.tile([P, CCH, 2, T], F32R, tag="qsb2")
                    ksb2 = wk_pool.tile([P, CCH, 2, T], F32R, tag="ksb2", bufs=1)
                    for wmat, bias_t, use_b, dst, eng in (
                        (wq, qb, use_qb, qsb2, "act"),
                        (wkk, kb, use_kb, ksb2, "dve"),
                    ):
                        for mc in range(CCH):
                            pp = ps1.tile([P, 2, T], F32, tag="ps1")
                            for kc in range(CCH):
                                nc.tensor.matmul(
                                    pp[:, :, :],
                                    wmat[:, kc, mc * P:(mc + 1) * P],
                                    xnT2[:, kc, :, :],
                                    start=(kc == 0), stop=(kc == CCH - 1),
                                )
                            if use_b:
                                nc.scalar.activation(
                                    dst[:, mc, :, :], pp[:], AF.Identity,
                                    bias=bias_t[:, mc:mc + 1])
                            elif eng == "act":
                                nc.scalar.copy(dst[:, mc, :, :], pp[:])
                            else:
                                nc.vector.tensor_copy(dst[:, mc, :, :], pp[:])

                    vsbs = []
                    for i in range(2):
                        vsb = wk_pool.tile([P, TCH, HD], F32R, tag=f"vsb{i}")
                        vsbs.append(vsb)
                        for sc in range(TCH):
                            vp = ps1.tile([P, HD], F32, tag="ps1")
                            for kc in range(CCH):
                                nc.tensor.matmul(
                                    vp[:, :],
                                    xnT2[:, kc, i, sc * P:(sc + 1) * P],
                                    wv[:, kc, :],
                                    start=(kc == 0),
                                    stop=(kc == CCH - 1 and not use_vb),
                                )
                            if use_vb:
                                nc.tensor.matmul(
                                    vp[:, :], ones_t[0:1, :], vb[0:1, :],
                                    start=False, stop=True)
                            if sc == 0:
                                nc.scalar.copy(vsb[:, sc, :], vp[:])
                            else:
                                nc.vector.tensor_copy(vsb[:, sc, :], vp[:])

                    # ---- attention per batch ----
                    xnews = []
                    for i, b in enumerate(bp):
                        e_all = bigp.tile([P, TCH, H, T], F32R, tag="e_all")
                        rbc = bigp.tile([P, H, T], F32, tag="rbc")
                        osb = wk_pool.tile([64, H, T], F32R, tag="osb", bufs=1)
                        for h in range(H):
                            hc, ho = h // 2, 64 * (h % 2)
                            sp = ps1.tile([P, TCH, T], F32, tag="ps1")
                            for sc in range(TCH):
                                nc.tensor.matmul(
                                    sp[:, sc, :],
                                    ksb2[ho:ho + D, hc, i, sc * P:(sc + 1) * P],
                                    qsb2[ho:ho + D, hc, i, :],
                                    start=True, stop=False,
                                )
                                # causal mask fused on PE: adds
                                # -1e30*max(0, s-t), so exp flushes to 0
                                nc.tensor.matmul(
                                    sp[:, sc, :], trilm[:, :], negm[:, sc, :],
                                    start=False, stop=True,
                                )
                            nc.scalar.activation(
                                e_all[:, :, h, :], sp[:], AF.Exp, scale=SCALE)
                        for pc in range(H // 2):
                            dp = ps1.tile([P, 2, T], F32, tag="ps1")
                            ops = []
                            for j in range(2):
                                h = 2 * pc + j
                                for sc in range(TCH):
                                    nc.tensor.matmul(
                                        dp[:, j, :], ones_t[:],
                                        e_all[:, sc, h, :],
                                        start=(sc == 0), stop=(sc == TCH - 1))
                                op_h = ps1.tile([D, T], F32, tag="ps1")
                                ops.append(op_h)
                                for sc in range(TCH):
                                    nc.tensor.matmul(
                                        op_h[:, :],
                                        vsbs[i][:, sc, h * D:(h + 1) * D],
                                        e_all[:, sc, h, :],
                                        start=(sc == 0), stop=(sc == TCH - 1))
                            nc.vector.reciprocal(
                                rbc[:, 2 * pc:2 * pc + 2, :], dp[:])
                            for j in range(2):
                                h = 2 * pc + j
                                nc.vector.tensor_tensor(
                                    osb[:, h, :], ops[j][:, :], rbc[0:D, h, :],
                                    ALU.mult,
                                )

                        # ---- out proj + residual ----
                        xnew = wk_pool.tile([P, TCH, C], F32, tag=f"xnew{i}")
                        xnews.append(xnew)
                        for tcc in range(TCH):
                            ap_t = ps1.tile([P, C], F32, tag="ps1")
                            for h in range(H):
                                nc.tensor.matmul(
                                    ap_t[:, :],
                                    osb[:, h, tcc * P:(tcc + 1) * P],
                                    wo[:, h, :],
                                    start=(h == 0),
                                    stop=(h == H - 1 and not use_bo))
                            if use_bo:
                                nc.tensor.matmul(
                                    ap_t[:, :], ones_t[0:1, :], bo[0:1, :],
                                    start=False, stop=True)
                            nc.vector.tensor_tensor(
                                xnew[:, tcc, :], ap_t[:, :], xts[i][:, tcc, :],
                                ALU.add)

                    # ---- LN2 -> xn2T pair ----
                    xn2T2 = wk_pool.tile([P, CCH, 2, T], F32R, tag="xn2T2")
                    for i in range(2):
                        layer_norm_T(xnews[i], xn2T2, i, evac_act=(i == 1))

                    # ---- FFN fused over the pair, streamed per f-chunk:
                    # hT chunk -> relu -> immediately accumulated into the
                    # four (batch, t-chunk) FFN2 output psums ----
                    fps = []
                    for j in range(4):
                        fp_j = ps1.tile([P, C], F32, tag="ps1", name=f"fp{j}")
                        fps.append(fp_j)
                    for mo in range(FCH):
                        hp = ps1.tile([P, 2, T], F32, tag="ps1")
                        for kc in range(CCH):
                            nc.tensor.matmul(
                                hp[:, :, :],
                                w1[:, kc, mo * P:(mo + 1) * P],
                                xn2T2[:, kc, :, :],
                                start=(kc == 0), stop=(kc == CCH - 1))
                        hsm = wk_pool.tile([P, 2, T], F32R, tag="hsm")
                        if use_b1:
                            nc.scalar.activation(
                                hsm[:], hp[:], AF.Relu, bias=b1c[:, mo:mo + 1])
                        else:
                            nc.scalar.activation(hsm[:], hp[:], AF.Relu)
                        for i in range(2):
                            for tcc in range(TCH):
                                nc.tensor.matmul(
                                    fps[2 * i + tcc][:, :],
                                    hsm[:, i, tcc * P:(tcc + 1) * P],
                                    w2[:, mo, :],
                                    start=(mo == 0),
                                    stop=(mo == FCH - 1 and not use_b2))

                    for i, b in enumerate(bp):
                        yout = wk_pool.tile([P, TCH, C], F32, tag=f"yout{i}")
                        for tcc in range(TCH):
                            fp = fps[2 * i + tcc]
                            if use_b2:
                                nc.tensor.matmul(
                                    fp[:, :], ones_t[0:1, :], b2[0:1, :],
                                    start=False, stop=True)
                            nc.vector.tensor_tensor(
                                yout[:, tcc, :], fp[:, :], xnews[i][:, tcc, :],
                                ALU.add)
                        nc.sync.dma_start(
                            y_d[b].rearrange("(tc p) c -> p tc c", p=P),
                            yout[:])

            if repeat > 1:
                with tc.For_i(0, repeat, 1):
                    body()
            else:
                body()

    nc.compile()
    return nc


def _make_negm():
    # negm[j, sc, t] moving operand; with trilm (lhsT[j, s] = 1 iff j <= s)
    # the accumulated matmul adds -BIG * #{j: j <= s_blk and cond(j, t)},
    # nonzero exactly where global s > t.
    BIG = np.float32(1e30)
    f32 = np.float32
    m = np.zeros((P, TCH, T), dtype=f32)
    jgt = np.tril(np.ones((P, P), dtype=f32), -1)  # [j, t] = 1 iff j > t
    m[:, 0, 0:P] = -BIG * jgt          # diagonal block of s-chunk 0
    m[:, 1, 0:P] = -BIG                # s-chunk 1 vs t-chunk 0: all masked
    m[:, 1, P:2 * P] = -BIG * jgt      # diagonal block of s-chunk 1
    return m


def prep_weights(Wq, Wk, Wv, Wo, bo, W1, b1, W2, b2, g1, be1, g2, be2):
    """Fold LN gamma/beta into projection weights; rearrange to SBUF layouts."""
    f32 = np.float32

    def kchunk(w, kdim):  # [K, M] -> [P, K//P, M]
        m = w.shape[1]
        return np.ascontiguousarray(
            w.reshape(kdim // P, P, m).transpose(1, 0, 2)).astype(f32)

    Wq2 = Wq.transpose(1, 0, 2).reshape(C, HD)
    Wk2 = Wk.transpose(1, 0, 2).reshape(C, HD)
    Wv2 = Wv.transpose(1, 0, 2).reshape(C, HD)
    out = {
        "wq": kchunk(g1[:, None] * Wq2, C),
        "wk": kchunk(g1[:, None] * Wk2, C),
        "wv": kchunk(g1[:, None] * Wv2, C),
        "wo": np.ascontiguousarray(
            Wo.reshape(H, D, C).transpose(1, 0, 2)).astype(f32),
        "w1": kchunk(g2[:, None] * W1, C),
        "w2": kchunk(W2, F),
        "ident": np.eye(P, dtype=f32),
        "onesm": np.ones((P, P), dtype=f32),
        "trilm": np.tril(np.ones((P, P), dtype=f32)).T.copy(),
        "negm": _make_negm(),
    }
    qb = be1 @ Wq2
    kb = be1 @ Wk2
    vb = be1 @ Wv2
    b1e = be2 @ W1 + b1
    out["qb"] = np.ascontiguousarray(qb.reshape(CCH, P).T).astype(f32)
    out["kb"] = np.ascontiguousarray(kb.reshape(CCH, P).T).astype(f32)
    out["vb"] = vb[None, :].astype(f32)
    out["bo"] = bo[None, :].astype(f32)
    out["b1c"] = np.ascontiguousarray(b1e.reshape(FCH, P).T).astype(f32)
    out["b2"] = b2[None, :].astype(f32)
    flags = set()
    for name, vec in (("qb", qb), ("kb", kb), ("vb", vb),
                      ("bo", bo), ("b1", b1e), ("b2", b2)):
        if np.any(vec != 0):
            flags.add(name)
    return out, frozenset(flags)


_PROGRAM_CACHE = {}


def _get_program(bl, flags):
    key = (bl, flags)
    if key not in _PROGRAM_CACHE:
        _PROGRAM_CACHE[key] = build_program(
            bl, flags, tr_split=True, ps1_bufs=8)
    return _PROGRAM_CACHE[key]


def kernel(x, Wq, Wk, Wv, Wo, bo, W1, b1, W2, b2, g1, be1, g2, be2, **kw):
    from concourse.bass_utils import run_bass_kernel_spmd

    args = [np.asarray(a, dtype=np.float32) for a in
            (x, Wq, Wk, Wv, Wo, bo, W1, b1, W2, b2, g1, be1, g2, be2)]
    x = args[0]
    wmap, flags = prep_weights(*args[1:])
    nc = _get_program(BL, flags)
    xs = x.reshape(NCORES, BL, T, C)
    in_maps = []
    for c in range(NCORES):
        m = {"x": np.ascontiguousarray(xs[c])}
        m.update(wmap)
        in_maps.append(m)
    res = run_bass_kernel_spmd(nc, in_maps, list(range(NCORES)), **kw)
    global _last_results
    _last_results = res
    y = np.stack([res.results[i]["y"] for i in range(NCORES)], axis=0)
    return y.reshape(B, T, C)


_last_results = None



# revision 26
# speedup vs baseline: 1.4925x; 1.4925x over previous
"""Trainium2 Bass kernel for a pre-LN transformer block (B=128,T=256,C=384,H=6,D=64).

Data-parallel over batch across 8 NeuronCores (16 batches/core). v2 fast path
(used when all bias/beta vectors are zero, which holds for this problem):

- every matmul runs in bf16 (stationary and moving), accumulating in f32 PSUM:
  1 PE cycle/row at any moving width, 1.0 (not 1.5) cycles/row for transposes.
- attention is split at the causal diagonal: the fully-masked
  (s-chunk 1, t-chunk 0) quadrant is never computed; causal masking adds
  -1e30*max(0,s-t) to the two diagonal 128x128 blocks via an extra matmul
  into the score psum.
- softmax denominators come from ones-matmuls over two heads at a time (which
  also broadcasts them across partitions); normalization and the residual adds
  run on the otherwise-idle Pool (gpsimd) engine.
- the output projection packs head pairs on partitions so its contraction uses
  the full K=128 array.
- stages are software-pipelined across batch pairs in emission order
  C(p), B(p+1), D(p), A(p+2), E(p) so LN chains for pair p+2 overlap
  attention/FFN of pair p.

The v1 (float32r) builder is kept as a fallback for nonzero biases.
"""

import sys

if "/opt/trn_rl_repo" not in sys.path:
    sys.path.insert(0, "/opt/trn_rl_repo")

import numpy as np

import concourse.bass as bass
import concourse.mybir as mybir
import concourse.tile as tile
from concourse import bacc

# All ACT functions used here (Exp, Ln, Relu, Identity, Copy) live in the
# 'natural_log_exp_and_others' table set. Blank the other sets (preserving
# dict order, which defines act_func_set_id) so the table-load fixpoint
# settles on a single ACT_TABLE_LOAD instead of thrashing sets per batch.
_KEEP_ACT_SET = "natural_log_exp_and_others"
_orig_get_act_tables = bacc.get_activation_tables


def _one_set_tables(arch):
    t = _orig_get_act_tables(arch)
    assert _KEEP_ACT_SET in t
    return {k: (v if k == _KEEP_ACT_SET else set()) for k, v in t.items()}


bacc.get_activation_tables = _one_set_tables

F32 = mybir.dt.float32
F32R = mybir.dt.float32r
BF16 = mybir.dt.bfloat16
AF = mybir.ActivationFunctionType
ALU = mybir.AluOpType

B, T, C, H, D = 128, 256, 384, 6, 64
NCORES = 8
BL = B // NCORES          # batches per core
F = 4 * C                 # 1536
P = 128
TCH = T // P              # 2 token chunks
CCH = C // P              # 3 channel chunks
FCH = F // P              # 12 ffn chunks
HP = H // 2               # head pairs
HD = H * D                # 384
SCALE = float(C) ** -0.5  # reference scales by full model dim
EPS = 1e-5


# --------------------------------------------------------------------------
# v2 builder: fp8 DoubleRow GEMMs, split-diagonal attention, pool offload,
# software pipeline.
#
# Scaling scheme: fp8 weights are stored x16 (lifting sigma~0.02 weights out
# of the e4m3 subnormal range). Activations carried unscaled except:
#   q,k carry x16 -> scores carry x256 -> folded into the exp scale
#   v carries x16 -> op/osb carry x16; wo2 is x16 -> ap carries x256
#     -> xnew = ap/256 + x via scalar_tensor_tensor
#   hsm carries x16 (relu of x16 psum), w2 x16 -> fps carry x256
#     -> y = fps/256 + xnew
# --------------------------------------------------------------------------

FP8 = mybir.dt.float8e4
WSC = 16.0                 # fp8 weight prescale
INV2 = 1.0 / (WSC * WSC)


def build_program_v2(bl=BL, repeat=1, ps1_bufs=4, debug_dump=False):
    assert bl % 2 == 0
    NP = bl // 2

    nc = bacc.Bacc("TRN2", target_bir_lowering=False, debug=False,
                   num_devices=NCORES)

    x_d = nc.dram_tensor("x", [bl, T, C], BF16, kind="ExternalInput")
    wq_d = nc.dram_tensor("wq", [P, CCH, HD], FP8, kind="ExternalInput")
    wk_d = nc.dram_tensor("wk", [P, CCH, HD], FP8, kind="ExternalInput")
    wv_d = nc.dram_tensor("wv", [P, CCH, HD], FP8, kind="ExternalInput")
    wo_d = nc.dram_tensor("wo2", [P, HP, C], FP8, kind="ExternalInput")
    w1_d = nc.dram_tensor("w1", [P, CCH, F], FP8, kind="ExternalInput")
    w2_d = nc.dram_tensor("w2", [P, FCH, C], FP8, kind="ExternalInput")
    id_d = nc.dram_tensor("identb", [P, P], BF16, kind="ExternalInput")
    i2_d = nc.dram_tensor("id256", [P, P], BF16, kind="ExternalInput")
    i6_d = nc.dram_tensor("id16i", [P, P], F32R, kind="ExternalInput")
    on_d = nc.dram_tensor("ones8", [P, P], FP8, kind="ExternalInput")
    tl_d = nc.dram_tensor("trilmb", [P, P], BF16, kind="ExternalInput")
    ng_d = nc.dram_tensor("negmb", [P, TCH, T], BF16, kind="ExternalInput")
    y_d = nc.dram_tensor("y", [bl, T, C], F32, kind="ExternalOutput")
    dbg = {}
    if debug_dump:
        dbg["xnT2"] = nc.dram_tensor("d_xnT2", [P, CCH, 2, T], F32,
                                     kind="ExternalOutput")
        dbg["qsb2"] = nc.dram_tensor("d_qsb2", [P, CCH, 2, T], F32,
                                     kind="ExternalOutput")
        dbg["ksb2"] = nc.dram_tensor("d_ksb2", [P, CCH, 2, T], F32,
                                     kind="ExternalOutput")
        dbg["vsb"] = nc.dram_tensor("d_vsb", [P, TCH, HD], F32,
                                    kind="ExternalOutput")
        dbg["e"] = nc.dram_tensor("d_e", [P, H, 2, T], F32,
                                  kind="ExternalOutput")
        dbg["rbc"] = nc.dram_tensor("d_rbc", [P, H, T], F32,
                                    kind="ExternalOutput")
        dbg["osb"] = nc.dram_tensor("d_osb", [P, HP, T], F32,
                                    kind="ExternalOutput")
        dbg["xnew"] = nc.dram_tensor("d_xnew", [P, TCH, C], F32,
                                     kind="ExternalOutput")
        dbg["vsb1"] = nc.dram_tensor("d_vsb1", [P, TCH, HD], F32,
                                     kind="ExternalOutput")
        dbg["e1"] = nc.dram_tensor("d_e1", [P, H, 2, T], F32,
                                   kind="ExternalOutput")
        dbg["rbc1"] = nc.dram_tensor("d_rbc1", [P, H, T], F32,
                                     kind="ExternalOutput")
        dbg["osb1"] = nc.dram_tensor("d_osb1", [P, HP, T], F32,
                                     kind="ExternalOutput")
        dbg["xnew1"] = nc.dram_tensor("d_xnew1", [P, TCH, C], F32,
                                      kind="ExternalOutput")
        dbg["xn2T2"] = nc.dram_tensor("d_xn2T2", [P, CCH, 2, T], F32,
                                      kind="ExternalOutput")
        dbg["hsm"] = nc.dram_tensor("d_hsm", [P, 2, 2, T], F32,
                                    kind="ExternalOutput")

    with tile.TileContext(nc) as tc:
        with (
            tc.tile_pool(name="wpool", bufs=1) as wp,
            tc.tile_pool(name="work", bufs=2) as wk,
            tc.tile_pool(name="ps1", bufs=ps1_bufs, space="PSUM") as ps1,
            tc.tile_pool(name="psb", bufs=1, space="PSUM") as psb,
        ):
            wq = wp.tile([P, CCH, HD], FP8)
            wkk = wp.tile([P, CCH, HD], FP8)
            wv = wp.tile([P, CCH, HD], FP8)
            wo2 = wp.tile([P, HP, C], FP8)
            w1 = wp.tile([P, CCH, F], FP8)
            w2 = wp.tile([P, FCH, C], FP8)
            identb = wp.tile([P, P], BF16)
            id256 = wp.tile([P, P], BF16)
            id16i = wp.tile([P, P], F32R)
            ones8 = wp.tile([P, P], FP8)
            trilmb = wp.tile([P, P], BF16)
            negmb = wp.tile([P, TCH, T], BF16)
            epsb = wp.tile([P, 1], F32)
            nc.gpsimd.memset(epsb[:], EPS)
            # identb first: the first PE work (LN1 transposes of pair 0)
            # needs only it and the x DMA, which stage_A(0) emits next.
            nc.sync.dma_start(identb[:], id_d[:])

            def load_weights():
                nc.sync.dma_start(wq[:], wq_d[:])
                nc.sync.dma_start(wkk[:], wk_d[:])
                nc.sync.dma_start(wv[:], wv_d[:])
                nc.sync.dma_start(trilmb[:], tl_d[:])
                nc.sync.dma_start(negmb[:], ng_d[:])
                nc.sync.dma_start(ones8[:], on_d[:])
                nc.sync.dma_start(wo2[:], wo_d[:])
                nc.sync.dma_start(id256[:], i2_d[:])
                nc.sync.dma_start(w1[:], w1_d[:])
                nc.sync.dma_start(w2[:], w2_d[:])
                nc.sync.dma_start(id16i[:], i6_d[:])

            DRm = mybir.MatmulPerfMode.DoubleRow

            def layer_norm_T(src, dstT, i, tagpfx, evac_act,
                             norm_eng="dve"):
                """src [P,TCH,C] (bf16 or f32r); (src-mu)*rstd -> bf16 ->
                transposed -> dstT[:, :, i, :] in fp8."""
                st6 = wk.tile([P, TCH, 6], F32, tag=f"{tagpfx}st6_{i}")
                mv = wk.tile([P, TCH, 2], F32, tag=f"{tagpfx}mv_{i}")
                rstd = wk.tile([P, TCH], F32, tag=f"{tagpfx}rstd_{i}")
                for tch in range(TCH):
                    nc.vector.bn_stats(st6[:, tch, :], src[:, tch, :])
                    nc.vector.bn_aggr(mv[:, tch, :], st6[:, tch, :])
                # rstd = exp(-0.5 * ln(var + eps))
                nc.scalar.activation(rstd[:], mv[:, :, 1], AF.Ln, bias=epsb[:])
                nc.scalar.activation(rstd[:], rstd[:], AF.Exp, scale=-0.5)
                xn = wk.tile([P, TCH, C], BF16, tag=f"{tagpfx}xn_{i}", bufs=1)
                neng = nc.gpsimd if norm_eng == "pool" else nc.vector
                for tch in range(TCH):
                    neng.tensor_scalar(
                        xn[:, tch, :], src[:, tch, :],
                        mv[:, tch, 0:1], rstd[:, tch:tch + 1],
                        ALU.subtract, ALU.mult,
                    )
                trAB = ps1.tile([P, CCH, T], BF16, tag="ps1", name="trAB")
                for tch in range(TCH):
                    for cc in range(CCH):
                        nc.tensor.transpose(
                            trAB[:, cc, tch * P:(tch + 1) * P],
                            xn[:, tch, cc * P:(cc + 1) * P], identb[:])
                if evac_act:
                    nc.scalar.copy(dstT[:, :, i, :], trAB[:])
                else:
                    nc.vector.tensor_copy(dstT[:, :, i, :], trAB[:])

            state = {}

            def dump(name, ap):
                if not debug_dump or name not in dbg:
                    return
                dt = wk.tile(list(ap.shape), F32, tag=f"dump_{name}", bufs=1)
                nc.vector.tensor_copy(dt[:], ap)
                nc.sync.dma_start(dbg.pop(name)[:], dt[:])

            def stage_A(p):
                """x DMA + LN1 + transposes -> xnT2(p)."""
                bp = (2 * p, 2 * p + 1)
                xts = []
                xnT2 = wk.tile([P, CCH, 2, T], FP8, tag="xnT2")
                for i, b in enumerate(bp):
                    xt = wk.tile([P, TCH, C], BF16, tag=f"xt{i}")
                    nc.sync.dma_start(
                        xt[:], x_d[b].rearrange("(tc p) c -> p tc c", p=P))
                    xts.append(xt)
                    layer_norm_T(xt, xnT2, i, "a", evac_act=(i == 0))
                if p == 0:
                    dump("xnT2", xnT2[:, :, :, :])
                state[p] = {"xts": xts, "xnT2": xnT2}

            def stage_B(p):
                """QKV projections from xnT2(p). q,k,v carry x16."""
                st = state[p]
                xnT2 = st["xnT2"]
                qsb2 = wk.tile([P, CCH, 2, T], BF16, tag="qsb2")
                ksb2 = wk.tile([P, CCH, 2, T], BF16, tag="ksb2")
                for wmat, dst, eng in ((wq, qsb2, "act"), (wkk, ksb2, "dve")):
                    for mc in range(CCH):
                        pp = ps1.tile([P, 2, T], F32, tag="ps1", name="pp")
                        nc.tensor.matmul(
                            pp[:, :, :],
                            wmat[:, 0:2, mc * P:(mc + 1) * P],
                            xnT2[:, 0:2, :, :],
                            start=True, stop=False, perf_mode=DRm)
                        nc.tensor.matmul(
                            pp[:, :, :],
                            wmat[:, 2, mc * P:(mc + 1) * P],
                            xnT2[:, 2, :, :],
                            start=False, stop=True)
                        if eng == "act":
                            nc.scalar.copy(dst[:, mc, :, :], pp[:])
                        else:
                            nc.vector.tensor_copy(dst[:, mc, :, :], pp[:])
                vsbs = []
                for i in range(2):
                    vsb = wk.tile([P, TCH, HD], FP8, tag=f"vsb{i}")
                    vsbs.append(vsb)
                    for sc in range(TCH):
                        vp = ps1.tile([P, HD], F32, tag="ps1", name="vp")
                        nc.tensor.matmul(
                            vp[:, :],
                            xnT2[:, 0:2, i, sc * P:(sc + 1) * P],
                            wv[:, 0:2, :],
                            start=True, stop=False, perf_mode=DRm)
                        nc.tensor.matmul(
                            vp[:, :],
                            xnT2[:, 2, i, sc * P:(sc + 1) * P],
                            wv[:, 2, :],
                            start=False, stop=True)
                        if sc == 0:
                            nc.scalar.copy(vsb[:, sc, :], vp[:])
                        else:
                            nc.vector.tensor_copy(vsb[:, sc, :], vp[:])
                if p == 0:
                    dump("qsb2", qsb2[:, :, :, :])
                    dump("ksb2", ksb2[:, :, :, :])
                    dump("vsb", vsbs[0][:, :, :])
                    dump("vsb1", vsbs[1][:, :, :])
                st["qsb2"] = qsb2
                st["ksb2"] = ksb2
                st["vsbs"] = vsbs

            def stage_C(p):
                """Attention + out-proj + residual -> xnews(p) (carrying
                x256)."""
                st = state[p]
                qsb2, ksb2, vsbs, xts = (st["qsb2"], st["ksb2"], st["vsbs"],
                                         st["xts"])
                osbs = []
                for i in range(2):
                    # e [P, H, sc, T] fp8; masked entries exp to exact 0
                    e_all = wk.tile([P, H, 2, T], FP8, tag=f"e{i}")
                    rbc = wk.tile([P, H, T], F32, tag=f"rbc{i}")
                    state[p][f"e{i}"] = e_all
                    state[p][f"rbc{i}"] = rbc
                    osb = wk.tile([P, HP, T], FP8, tag=f"osb{i}", bufs=1)
                    osbs.append(osb)
                    for j in range(HP):  # head pair
                        for j2 in range(2):
                            h = 2 * j + j2
                            hc, ho = h // 2, 64 * (h % 2)
                            kap = ksb2[ho:ho + D, hc, i, :]
                            qap = qsb2[ho:ho + D, hc, i, :]
                            sp = ps1.tile([P, 2, T], F32, tag="ps1",
                                          name="sp")
                            for sc in range(TCH):
                                nc.tensor.matmul(
                                    sp[:, sc, :],
                                    kap[:, sc * P:(sc + 1) * P], qap[:, :],
                                    start=True, stop=False,
                                    skip_group_check=True)
                            nc.tensor.matmul(
                                sp[:, :, :], trilmb[:], negmb[:, :, :],
                                start=False, stop=True, skip_group_check=True)
                            # q,k carry x16 each -> scores carry x256
                            nc.scalar.activation(
                                e_all[:, h, :, :], sp[:], AF.Exp,
                                scale=SCALE * INV2)
                        # denominators for the pair (broadcast over
                        # partitions); skip the all-masked quadrant
                        dp = ps1.tile([P, 2, T], F32, tag="ps1", name="dp")
                        nc.tensor.matmul(
                            dp[:, :, :], ones8[:],
                            e_all[:, 2 * j:2 * j + 2, 0, :],
                            start=True, stop=False)
                        nc.tensor.matmul(
                            dp[:, :, 128:256], ones8[:],
                            e_all[:, 2 * j:2 * j + 2, 1, 128:256],
                            start=False, stop=True, skip_group_check=True)
                        nc.vector.reciprocal_approx_fast(
                            rbc[:, 2 * j:2 * j + 2, :], dp[:])
                        # attn @ v, both s-chunks in one DoubleRow per head
                        op2 = ps1.tile([D, 2, T], F32, tag="ps1", name="op2")
                        for j2 in range(2):
                            h = 2 * j + j2
                            nc.tensor.matmul(
                                op2[:, j2, :],
                                vsbs[i][:, :, h * D:(h + 1) * D],
                                e_all[:, h, :, :],
                                start=True, stop=True, perf_mode=DRm,
                                skip_group_check=True)
                        for j2 in range(2):
                            h = 2 * j + j2
                            nc.vector.tensor_tensor(
                                osb[j2 * D:(j2 + 1) * D, j, :],
                                op2[0:D, j2, :],
                                rbc[j2 * D:(j2 + 1) * D, h, :], ALU.mult)

                if p == 0:
                    dump("e", state[p]["e0"][:, :, :, :])
                    dump("rbc", state[p]["rbc0"][:, :, :])
                    dump("osb", osbs[0][:, :, :])
                    dump("e1", state[p]["e1"][:, :, :, :])
                    dump("rbc1", state[p]["rbc1"][:, :, :])
                    dump("osb1", osbs[1][:, :, :])
                # out-proj (head pairs packed on partitions) + residual via
                # scaled-identity matmul; evacuated as one copy per batch
                xnews = []
                for i in range(2):
                    # 512-wide slots keep each matmul output inside one
                    # 2KB PSUM bank
                    ap2 = psb.tile([P, 4, 512], F32, tag="big", name="ap2")
                    for tcc in range(TCH):
                        nc.tensor.matmul(
                            ap2[:, tcc, 0:C],
                            osbs[i][:, 0:2, tcc * P:(tcc + 1) * P],
                            wo2[:, 0:2, :],
                            start=True, stop=False, perf_mode=DRm,
                            skip_group_check=True)
                        nc.tensor.matmul(
                            ap2[:, tcc, 0:C],
                            osbs[i][:, 2, tcc * P:(tcc + 1) * P],
                            wo2[:, 2, :],
                            start=False, stop=False, skip_group_check=True)
                        # + 256*x residual (osb x16 * wo2 x16 = x256 domain)
                        nc.tensor.matmul(
                            ap2[:, tcc, 0:C], id256[:], xts[i][:, tcc, :],
                            start=False, stop=True, skip_group_check=True)
                    xnew = wk.tile([P, TCH, C], F32R, tag=f"xnew{i}")
                    xnews.append(xnew)
                    if i == 0:
                        nc.scalar.copy(xnew[:, :, :], ap2[:, 0:TCH, 0:C])
                    else:
                        nc.vector.tensor_copy(xnew[:, :, :],
                                              ap2[:, 0:TCH, 0:C])
                if p == 0:
                    dump("xnew", xnews[0][:, :, :])
                    dump("xnew1", xnews[1][:, :, :])
                st["xnews"] = xnews

            def stage_D(p):
                """LN2 + transposes -> xn2T2(p). LN is scale-invariant, so
                the x256 carried by xnew drops out here."""
                st = state[p]
                xn2T2 = wk.tile([P, CCH, 2, T], FP8, tag="xn2T2")
                for i in range(2):
                    layer_norm_T(st["xnews"][i], xn2T2, i, "d",
                                 evac_act=(i == 1))
                if p == 0:
                    dump("xn2T2", xn2T2[:, :, :, :])
                st["xn2T2"] = xn2T2

            def stage_E(p):
                """FFN + residual + store. hsm true scale, w2 x16."""
                st = state[p]
                bp = (2 * p, 2 * p + 1)
                xn2T2 = st["xn2T2"]
                fpsb = psb.tile([P, 4, 512], F32, tag="big", name="fpsb")

                def fslot(i, tcc):
                    return fpsb[:, 2 * i + tcc, 0:C]

                for mp in range(FCH // 2):
                    # hsm [P, mo-sub, batch, T]: mo-subtile dim is the
                    # DoubleRow k-pair for FFN2
                    hsm = wk.tile([P, 2, 2, T], FP8, tag="hsm", bufs=3)
                    for moj in range(2):
                        mo = 2 * mp + moj
                        hp = ps1.tile([P, 2, T], F32, tag="ps1", name="hp")
                        nc.tensor.matmul(
                            hp[:, :, :],
                            w1[:, 0:2, mo * P:(mo + 1) * P],
                            xn2T2[:, 0:2, :, :],
                            start=True, stop=False, perf_mode=DRm)
                        nc.tensor.matmul(
                            hp[:, :, :],
                            w1[:, 2, mo * P:(mo + 1) * P],
                            xn2T2[:, 2, :, :],
                            start=False, stop=True)
                        # hp carries x16 -> scale back to true h
                        if moj == 0:
                            nc.scalar.activation(
                                hsm[:, moj, :, :], hp[:], AF.Relu,
                                scale=1.0 / WSC)
                        else:
                            nc.vector.tensor_scalar(
                                hsm[:, moj, :, :], hp[:], 1.0 / WSC, 0.0,
                                ALU.mult, ALU.max)
                    if p == 0 and mp == 0:
                        dump("hsm", hsm[:, :, :, :])
                    for i in range(2):
                        for tcc in range(TCH):
                            nc.tensor.matmul(
                                fslot(i, tcc),
                                hsm[:, :, i, tcc * P:(tcc + 1) * P],
                                w2[:, 2 * mp:2 * mp + 2, :],
                                start=(mp == 0), stop=False,
                                perf_mode=DRm, skip_group_check=True)
                # fps carries x16 (h * 16w2); add xnew256/16 so one /16
                # evacuation yields y
                for i in range(2):
                    for tcc in range(TCH):
                        nc.tensor.matmul(
                            fslot(i, tcc), id16i[:],
                            st["xnews"][i][:, tcc, :],
                            start=False, stop=True, skip_group_check=True)
                yout = wk.tile([P, 4, C], F32, tag="yout")
                nc.scalar.activation(yout[:], fpsb[:, :, 0:C], AF.Copy,
                                     scale=1.0 / WSC)
                for i, b in enumerate(bp):
                    nc.sync.dma_start(
                        y_d[b].rearrange("(tc p) c -> p tc c", p=P),
                        yout[:, 2 * i:2 * i + 2, :])

            def body(emit_weights):
                state.clear()
                stage_A(0)
                if emit_weights:
                    load_weights()
                stage_B(0)
                if NP > 1:
                    stage_A(1)
                for p in range(NP):
                    stage_C(p)
                    if p + 1 < NP:
                        stage_B(p + 1)
                    stage_D(p)
                    if p + 2 < NP:
                        stage_A(p + 2)
                    stage_E(p)
                    if p - 1 in state:
                        del state[p - 1]

            if repeat > 1:
                load_weights()
                with tc.For_i(0, repeat, 1):
                    body(emit_weights=False)
            else:
                body(emit_weights=True)

    nc.compile()
    return nc


def prep_weights_v2(Wq, Wk, Wv, Wo, bo, W1, b1, W2, b2, g1, be1, g2, be2):
    import ml_dtypes
    bf = ml_dtypes.bfloat16
    f8 = ml_dtypes.float8_e4m3
    f32 = np.float32

    def kchunk(w, kdim):  # [K, M] -> [P, K//P, M], x16 in fp8
        m = w.shape[1]
        return np.ascontiguousarray(
            (WSC * w).reshape(kdim // P, P, m).transpose(1, 0, 2)).astype(f8)

    Wq2 = Wq.transpose(1, 0, 2).reshape(C, HD)
    Wk2 = Wk.transpose(1, 0, 2).reshape(C, HD)
    Wv2 = Wv.transpose(1, 0, 2).reshape(C, HD)
    WoR = Wo.reshape(H, D, C)
    wo2 = np.zeros((P, HP, C), dtype=f32)
    for h in range(H):
        wo2[64 * (h % 2):64 * (h % 2) + 64, h // 2, :] = WoR[h]
    out = {
        "wq": kchunk(g1[:, None] * Wq2, C),
        "wk": kchunk(g1[:, None] * Wk2, C),
        "wv": kchunk(g1[:, None] * Wv2, C),
        "wo2": (WSC * wo2).astype(f8),
        "w1": kchunk(g2[:, None] * W1, C),
        "w2": kchunk(W2, F),
        "identb": np.eye(P, dtype=f32).astype(bf),
        "id256": (256.0 * np.eye(P, dtype=f32)).astype(bf),
        "id16i": (np.eye(P, dtype=f32) / 16.0),
        "ones8": np.ones((P, P), dtype=f32).astype(f8),
        "trilmb": np.tril(np.ones((P, P), dtype=f32)).T.copy().astype(bf),
        "negmb": _make_negm().astype(bf),
    }
    return out


# --------------------------------------------------------------------------
# v1 builder (float32r, kept as fallback for nonzero biases)
# --------------------------------------------------------------------------

def build_program(bl=BL, flags=frozenset(), repeat=1,
                  tr_split=False, tr_bufs=1, ps1_bufs=6, wk_bufs=2):
    """Per-core Bass program. `flags` lists nonzero bias terms
    ('qb','kb','vb','bo','b1','b2'). `repeat` wraps the whole computation in
    a hardware loop (benchmarking only)."""
    assert bl % 2 == 0
    use_qb = "qb" in flags
    use_kb = "kb" in flags
    use_vb = "vb" in flags
    use_bo = "bo" in flags
    use_b1 = "b1" in flags
    use_b2 = "b2" in flags

    nc = bacc.Bacc("TRN2", target_bir_lowering=False, debug=False,
                   num_devices=NCORES)

    x_d = nc.dram_tensor("x", [bl, T, C], F32, kind="ExternalInput")
    wq_d = nc.dram_tensor("wq", [P, CCH, HD], F32R, kind="ExternalInput")
    wk_d = nc.dram_tensor("wk", [P, CCH, HD], F32R, kind="ExternalInput")
    wv_d = nc.dram_tensor("wv", [P, CCH, HD], F32R, kind="ExternalInput")
    qb_d = nc.dram_tensor("qb", [P, CCH], F32, kind="ExternalInput")
    kb_d = nc.dram_tensor("kb", [P, CCH], F32, kind="ExternalInput")
    vb_d = nc.dram_tensor("vb", [1, HD], F32R, kind="ExternalInput")
    wo_d = nc.dram_tensor("wo", [D, H, C], F32R, kind="ExternalInput")
    bo_d = nc.dram_tensor("bo", [1, C], F32R, kind="ExternalInput")
    w1_d = nc.dram_tensor("w1", [P, CCH, F], F32R, kind="ExternalInput")
    b1_d = nc.dram_tensor("b1c", [P, FCH], F32, kind="ExternalInput")
    w2_d = nc.dram_tensor("w2", [P, FCH, C], F32R, kind="ExternalInput")
    b2_d = nc.dram_tensor("b2", [1, C], F32R, kind="ExternalInput")
    id_d = nc.dram_tensor("ident", [P, P], F32R, kind="ExternalInput")
    on_d = nc.dram_tensor("onesm", [P, P], F32R, kind="ExternalInput")
    tl_d = nc.dram_tensor("trilm", [P, P], F32R, kind="ExternalInput")
    ng_d = nc.dram_tensor("negm", [P, TCH, T], F32R, kind="ExternalInput")
    y_d = nc.dram_tensor("y", [bl, T, C], F32, kind="ExternalOutput")
    dbg = {}
    if debug_dump:
        dbg["xnT2"] = nc.dram_tensor("d_xnT2", [P, CCH, 2, T], F32,
                                     kind="ExternalOutput")
        dbg["qsb2"] = nc.dram_tensor("d_qsb2", [P, CCH, 2, T], F32,
                                     kind="ExternalOutput")
        dbg["ksb2"] = nc.dram_tensor("d_ksb2", [P, CCH, 2, T], F32,
                                     kind="ExternalOutput")
        dbg["vsb"] = nc.dram_tensor("d_vsb", [P, TCH, HD], F32,
                                    kind="ExternalOutput")
        dbg["e"] = nc.dram_tensor("d_e", [P, H, 2, T], F32,
                                  kind="ExternalOutput")
        dbg["rbc"] = nc.dram_tensor("d_rbc", [P, H, T], F32,
                                    kind="ExternalOutput")
        dbg["osb"] = nc.dram_tensor("d_osb", [P, HP, T], F32,
                                    kind="ExternalOutput")
        dbg["xnew"] = nc.dram_tensor("d_xnew", [P, TCH, C], F32,
                                     kind="ExternalOutput")
        dbg["vsb1"] = nc.dram_tensor("d_vsb1", [P, TCH, HD], F32,
                                     kind="ExternalOutput")
        dbg["e1"] = nc.dram_tensor("d_e1", [P, H, 2, T], F32,
                                   kind="ExternalOutput")
        dbg["rbc1"] = nc.dram_tensor("d_rbc1", [P, H, T], F32,
                                     kind="ExternalOutput")
        dbg["osb1"] = nc.dram_tensor("d_osb1", [P, HP, T], F32,
                                     kind="ExternalOutput")
        dbg["xnew1"] = nc.dram_tensor("d_xnew1", [P, TCH, C], F32,
                                      kind="ExternalOutput")
        dbg["xn2T2"] = nc.dram_tensor("d_xn2T2", [P, CCH, 2, T], F32,
                                      kind="ExternalOutput")
        dbg["hsm"] = nc.dram_tensor("d_hsm", [P, 2, 2, T], F32,
                                    kind="ExternalOutput")

    with tile.TileContext(nc) as tc:
        import contextlib
        with (
            tc.tile_pool(name="wpool", bufs=1) as wp,
            tc.tile_pool(name="work", bufs=wk_bufs) as wk_pool,
            tc.tile_pool(name="big", bufs=1) as bigp,
            (contextlib.nullcontext(None) if tr_split else
             tc.tile_pool(name="ps_tr", bufs=tr_bufs, space="PSUM")) as pstr,
            tc.tile_pool(name="ps_one", bufs=ps1_bufs, space="PSUM") as ps1,
        ):
            # ---- load weights/constants once ----
            wq = wp.tile([P, CCH, HD], F32R)
            wkk = wp.tile([P, CCH, HD], F32R)
            wv = wp.tile([P, CCH, HD], F32R)
            wo = wp.tile([D, H, C], F32R)
            w1 = wp.tile([P, CCH, F], F32R)
            w2 = wp.tile([P, FCH, C], F32R)
            ident = wp.tile([P, P], F32R)
            ones_t = wp.tile([P, P], F32R)
            trilm = wp.tile([P, P], F32R)
            negm = wp.tile([P, TCH, T], F32R)
            epsb = wp.tile([P, 1], F32)
            nc.gpsimd.memset(epsb[:], EPS)
            nc.sync.dma_start(wq[:], wq_d[:])
            nc.sync.dma_start(wkk[:], wk_d[:])
            nc.sync.dma_start(wv[:], wv_d[:])
            nc.sync.dma_start(wo[:], wo_d[:])
            nc.sync.dma_start(w1[:], w1_d[:])
            nc.sync.dma_start(w2[:], w2_d[:])
            nc.sync.dma_start(ident[:], id_d[:])
            nc.sync.dma_start(ones_t[:], on_d[:])
            nc.sync.dma_start(trilm[:], tl_d[:])
            nc.sync.dma_start(negm[:], ng_d[:])
            qb = kb = vb = bo = b1c = b2 = None
            if use_qb:
                qb = wp.tile([P, CCH], F32)
                nc.sync.dma_start(qb[:], qb_d[:])
            if use_kb:
                kb = wp.tile([P, CCH], F32)
                nc.sync.dma_start(kb[:], kb_d[:])
            if use_vb:
                vb = wp.tile([1, HD], F32R)
                nc.sync.dma_start(vb[:], vb_d[:])
            if use_bo:
                bo = wp.tile([1, C], F32R)
                nc.sync.dma_start(bo[:], bo_d[:])
            if use_b1:
                b1c = wp.tile([P, FCH], F32)
                nc.sync.dma_start(b1c[:], b1_d[:])
            if use_b2:
                b2 = wp.tile([1, C], F32R)
                nc.sync.dma_start(b2[:], b2_d[:])

            def layer_norm_T(src, dstT, i, evac_act):
                st6 = wk_pool.tile([P, TCH, 6], F32, tag=f"st6_{i}")
                mv = wk_pool.tile([P, TCH, 2], F32, tag=f"mv_{i}")
                rstd = wk_pool.tile([P, TCH], F32, tag=f"rstd_{i}")
                for tch in range(TCH):
                    nc.vector.bn_stats(st6[:, tch, :], src[:, tch, :])
                    nc.vector.bn_aggr(mv[:, tch, :], st6[:, tch, :])
                nc.scalar.activation(rstd[:], mv[:, :, 1], AF.Ln, bias=epsb[:])
                nc.scalar.activation(rstd[:], rstd[:], AF.Exp, scale=-0.5)
                xn = wk_pool.tile([P, TCH, C], F32R, tag=f"xn_{i}", bufs=1)
                for tch in range(TCH):
                    nc.vector.tensor_scalar(
                        xn[:, tch, :], src[:, tch, :],
                        mv[:, tch, 0:1], rstd[:, tch:tch + 1],
                        ALU.subtract, ALU.mult,
                    )
                trA = ps1.tile([P, 2, T], F32R, tag="ps1", name="trA")
                trB = ps1.tile([P, T], F32R, tag="ps1", name="trB")

                def _trdst(cc):
                    return trB if cc == 2 else trA[:, cc, :]
                for tch in range(TCH):
                    for cc in range(CCH):
                        nc.tensor.transpose(
                            _trdst(cc)[:, tch * P:(tch + 1) * P],
                            xn[:, tch, cc * P:(cc + 1) * P],
                            ident[:],
                        )
                if evac_act:
                    nc.scalar.copy(dstT[:, 0:2, i, :], trA[:])
                    nc.scalar.copy(dstT[:, 2, i, :], trB[:])
                else:
                    nc.vector.tensor_copy(dstT[:, 0:2, i, :], trA[:])
                    nc.vector.tensor_copy(dstT[:, 2, i, :], trB[:])

            def body():
                for pb in range(bl // 2):
                    bp = (2 * pb, 2 * pb + 1)
                    xts = []
                    xnT2 = wk_pool.tile([P, CCH, 2, T], F32R, tag="xnT2")
                    for i, b in enumerate(bp):
                        xt = wk_pool.tile([P, TCH, C], F32, tag=f"xt{i}")
                        nc.sync.dma_start(
                            xt[:], x_d[b].rearrange("(tc p) c -> p tc c", p=P))
                        xts.append(xt)
                        layer_norm_T(xt, xnT2, i, evac_act=(i == 0))

                    qsb2 = wk_pool.tile([P, CCH, 2, T], F32R, tag="qsb2")
                    ksb2 = wk_pool.tile([P, CCH, 2, T], F32R, tag="ksb2", bufs=1)
                    for wmat, bias_t, use_b, dst, eng in (
                        (wq, qb, use_qb, qsb2, "act"),
                        (wkk, kb, use_kb, ksb2, "dve"),
                    ):
                        for mc in range(CCH):
                            pp = ps1.tile([P, 2, T], F32, tag="ps1")
                            for kc in range(CCH):
                                nc.tensor.matmul(
                                    pp[:, :, :],
                                    wmat[:, kc, mc * P:(mc + 1) * P],
                                    xnT2[:, kc, :, :],
                                    start=(kc == 0), stop=(kc == CCH - 1),
                                )
                            if use_b:
                                nc.scalar.activation(
                                    dst[:, mc, :, :], pp[:], AF.Identity,
                                    bias=bias_t[:, mc:mc + 1])
                            elif eng == "act":
                                nc.scalar.copy(dst[:, mc, :, :], pp[:])
                            else:
                                nc.vector.tensor_copy(dst[:, mc, :, :], pp[:])

                    vsbs = []
                    for i in range(2):
                        vsb = wk_pool.tile([P, TCH, HD], F32R, tag=f"vsb{i}")
                        vsbs.append(vsb)
                        for sc in range(TCH):
                            vp = ps1.tile([P, HD], F32, tag="ps1")
                            for kc in range(CCH):
                                nc.tensor.matmul(
                                    vp[:, :],
                                    xnT2[:, kc, i, sc * P:(sc + 1) * P],
                                    wv[:, kc, :],
                                    start=(kc == 0),
                                    stop=(kc == CCH - 1 and not use_vb),
                                )
                            if use_vb:
                                nc.tensor.matmul(
                                    vp[:, :], ones_t[0:1, :], vb[0:1, :],
                                    start=False, stop=True)
                            if sc == 0:
                                nc.scalar.copy(vsb[:, sc, :], vp[:])
                            else:
                                nc.vector.tensor_copy(vsb[:, sc, :], vp[:])

                    xnews = []
                    for i, b in enumerate(bp):
                        e_all = bigp.tile([P, TCH, H, T], F32R, tag="e_all")
                        rbc = bigp.tile([P, H, T], F32, tag="rbc")
                        osb = wk_pool.tile([64, H, T], F32R, tag="osb", bufs=1)
                        for h in range(H):
                            hc, ho = h // 2, 64 * (h % 2)
                            sp = ps1.tile([P, TCH, T], F32, tag="ps1")
                            for sc in range(TCH):
                                nc.tensor.matmul(
                                    sp[:, sc, :],
                                    ksb2[ho:ho + D, hc, i, sc * P:(sc + 1) * P],
                                    qsb2[ho:ho + D, hc, i, :],
                                    start=True, stop=False,
                                )
                                nc.tensor.matmul(
                                    sp[:, sc, :], trilm[:, :], negm[:, sc, :],
                                    start=False, stop=True,
                                )
                            nc.scalar.activation(
                                e_all[:, :, h, :], sp[:], AF.Exp, scale=SCALE)
                        for pc in range(H // 2):
                            dp = ps1.tile([P, 2, T], F32, tag="ps1")
                            ops = []
                            for j in range(2):
                                h = 2 * pc + j
                                for sc in range(TCH):
                                    nc.tensor.matmul(
                                        dp[:, j, :], ones_t[:],
                                        e_all[:, sc, h, :],
                                        start=(sc == 0), stop=(sc == TCH - 1))
                                op_h = ps1.tile([D, T], F32, tag="ps1")
                                ops.append(op_h)
                                for sc in range(TCH):
                                    nc.tensor.matmul(
                                        op_h[:, :],
                                        vsbs[i][:, sc, h * D:(h + 1) * D],
                                        e_all[:, sc, h, :],
                                        start=(sc == 0), stop=(sc == TCH - 1))
                            nc.vector.reciprocal(
                                rbc[:, 2 * pc:2 * pc + 2, :], dp[:])
                            for j in range(2):
                                h = 2 * pc + j
                                nc.vector.tensor_tensor(
                                    osb[:, h, :], ops[j][:, :], rbc[0:D, h, :],
                                    ALU.mult,
                                )

                        xnew = wk_pool.tile([P, TCH, C], F32, tag=f"xnew{i}")
                        xnews.append(xnew)
                        for tcc in range(TCH):
                            ap_t = ps1.tile([P, C], F32, tag="ps1")
                            for h in range(H):
                                nc.tensor.matmul(
                                    ap_t[:, :],
                                    osb[:, h, tcc * P:(tcc + 1) * P],
                                    wo[:, h, :],
                                    start=(h == 0),
                                    stop=(h == H - 1 and not use_bo))
                            if use_bo:
                                nc.tensor.matmul(
                                    ap_t[:, :], ones_t[0:1, :], bo[0:1, :],
                                    start=False, stop=True)
                            nc.vector.tensor_tensor(
                                xnew[:, tcc, :], ap_t[:, :], xts[i][:, tcc, :],
                                ALU.add)

                    xn2T2 = wk_pool.tile([P, CCH, 2, T], F32R, tag="xn2T2")
                    for i in range(2):
                        layer_norm_T(xnews[i], xn2T2, i, evac_act=(i == 1))

                    fps = []
                    for j in range(4):
                        fp_j = ps1.tile([P, C], F32, tag="ps1", name=f"fp{j}")
                        fps.append(fp_j)
                    for mo in range(FCH):
                        hp = ps1.tile([P, 2, T], F32, tag="ps1")
                        for kc in range(CCH):
                            nc.tensor.matmul(
                                hp[:, :, :],
                                w1[:, kc, mo * P:(mo + 1) * P],
                                xn2T2[:, kc, :, :],
                                start=(kc == 0), stop=(kc == CCH - 1))
                        hsm = wk_pool.tile([P, 2, T], F32R, tag="hsm")
                        if use_b1:
                            nc.scalar.activation(
                                hsm[:], hp[:], AF.Relu, bias=b1c[:, mo:mo + 1])
                        else:
                            nc.scalar.activation(hsm[:], hp[:], AF.Relu)
                        for i in range(2):
                            for tcc in range(TCH):
                                nc.tensor.matmul(
                                    fps[2 * i + tcc][:, :],
                                    hsm[:, i, tcc * P:(tcc + 1) * P],
                                    w2[:, mo, :],
                                    start=(mo == 0),
                                    stop=(mo == FCH - 1 and not use_b2))

                    for i, b in enumerate(bp):
                        yout = wk_pool.tile([P, TCH, C], F32, tag=f"yout{i}")
                        for tcc in range(TCH):
                            fp = fps[2 * i + tcc]
                            if use_b2:
                                nc.tensor.matmul(
                                    fp[:, :], ones_t[0:1, :], b2[0:1, :],
                                    start=False, stop=True)
                            nc.vector.tensor_tensor(
                                yout[:, tcc, :], fp[:, :], xnews[i][:, tcc, :],
                                ALU.add)
                        nc.sync.dma_start(
                            y_d[b].rearrange("(tc p) c -> p tc c", p=P),
                            yout[:])

            if repeat > 1:
                with tc.For_i(0, repeat, 1):
                    body()
            else:
                body()

    nc.compile()
    return nc


def _make_negm():
    BIG = np.float32(1e30)
    f32 = np.float32
    m = np.zeros((P, TCH, T), dtype=f32)
    jgt = np.tril(np.ones((P, P), dtype=f32), -1)  # [j, t] = 1 iff j > t
    m[:, 0, 0:P] = -BIG * jgt
    m[:, 1, 0:P] = -BIG
    m[:, 1, P:2 * P] = -BIG * jgt
    return m


def prep_weights(Wq, Wk, Wv, Wo, bo, W1, b1, W2, b2, g1, be1, g2, be2):
    """Fold LN gamma/beta into projection weights; rearrange to SBUF layouts."""
    f32 = np.float32

    def kchunk(w, kdim):  # [K, M] -> [P, K//P, M]
        m = w.shape[1]
        return np.ascontiguousarray(
            w.reshape(kdim // P, P, m).transpose(1, 0, 2)).astype(f32)

    Wq2 = Wq.transpose(1, 0, 2).reshape(C, HD)
    Wk2 = Wk.transpose(1, 0, 2).reshape(C, HD)
    Wv2 = Wv.transpose(1, 0, 2).reshape(C, HD)
    out = {
        "wq": kchunk(g1[:, None] * Wq2, C),
        "wk": kchunk(g1[:, None] * Wk2, C),
        "wv": kchunk(g1[:, None] * Wv2, C),
        "wo": np.ascontiguousarray(
            Wo.reshape(H, D, C).transpose(1, 0, 2)).astype(f32),
        "w1": kchunk(g2[:, None] * W1, C),
        "w2": kchunk(W2, F),
        "ident": np.eye(P, dtype=f32),
        "onesm": np.ones((P, P), dtype=f32),
        "trilm": np.tril(np.ones((P, P), dtype=f32)).T.copy(),
        "negm": _make_negm(),
    }
    qb = be1 @ Wq2
    kb = be1 @ Wk2
    vb = be1 @ Wv2
    b1e = be2 @ W1 + b1
    out["qb"] = np.ascontiguousarray(qb.reshape(CCH, P).T).astype(f32)
    out["kb"] = np.ascontiguousarray(kb.reshape(CCH, P).T).astype(f32)
    out["vb"] = vb[None, :].astype(f32)
    out["bo"] = bo[None, :].astype(f32)
    out["b1c"] = np.ascontiguousarray(b1e.reshape(FCH, P).T).astype(f32)
    out["b2"] = b2[None, :].astype(f32)
    flags = set()
    for name, vec in (("qb", qb), ("kb", kb), ("vb", vb),
                      ("bo", bo), ("b1", b1e), ("b2", b2)):
        if np.any(vec != 0):
            flags.add(name)
    return out, frozenset(flags)


_PROGRAM_CACHE = {}


def _get_program(bl, flags):
    key = (bl, flags)
    if key not in _PROGRAM_CACHE:
        if flags:
            _PROGRAM_CACHE[key] = build_program(
                bl, flags, tr_split=True, ps1_bufs=8)
        else:
            _PROGRAM_CACHE[key] = build_program_v2(bl)
    return _PROGRAM_CACHE[key]


def kernel(x, Wq, Wk, Wv, Wo, bo, W1, b1, W2, b2, g1, be1, g2, be2, **kw):
    from concourse.bass_utils import run_bass_kernel_spmd

    import ml_dtypes

    args = [np.asarray(a, dtype=np.float32) for a in
            (x, Wq, Wk, Wv, Wo, bo, W1, b1, W2, b2, g1, be1, g2, be2)]
    x = args[0]
    _, flags = _probe_flags(*args[1:])
    if flags:
        wmap, flags = prep_weights(*args[1:])
    else:
        wmap = prep_weights_v2(*args[1:])
        x = x.astype(ml_dtypes.bfloat16)
    nc = _get_program(BL, flags)
    xs = x.reshape(NCORES, BL, T, C)
    in_maps = []
    for c in range(NCORES):
        m = {"x": np.ascontiguousarray(xs[c])}
        m.update(wmap)
        in_maps.append(m)
    res = run_bass_kernel_spmd(nc, in_maps, list(range(NCORES)), **kw)
    global _last_results
    _last_results = res
    y = np.stack([res.results[i]["y"] for i in range(NCORES)], axis=0)
    return y.reshape(B, T, C)


def _probe_flags(Wq, Wk, Wv, Wo, bo, W1, b1, W2, b2, g1, be1, g2, be2):
    qb = be1 @ Wq.transpose(1, 0, 2).reshape(C, HD)
    kb = be1 @ Wk.transpose(1, 0, 2).reshape(C, HD)
    vb = be1 @ Wv.transpose(1, 0, 2).reshape(C, HD)
    b1e = be2 @ W1 + b1
    flags = set()
    for name, vec in (("qb", qb), ("kb", kb), ("vb", vb),
                      ("bo", bo), ("b1", b1e), ("b2", b2)):
        if np.any(vec != 0):
            flags.add(name)
    return None, frozenset(flags)


_last_results = None


# revision 27
# speedup vs baseline: 1.6711x; 1.1196x over previous
"""Trainium2 Bass kernel for a pre-LN transformer block (B=128,T=256,C=384,H=6,D=64).

Data-parallel over batch across 8 NeuronCores (16 batches/core). v2 fast path
(used when all bias/beta vectors are zero, which holds for this problem):

- every matmul runs in bf16 (stationary and moving), accumulating in f32 PSUM:
  1 PE cycle/row at any moving width, 1.0 (not 1.5) cycles/row for transposes.
- attention is split at the causal diagonal: the fully-masked
  (s-chunk 1, t-chunk 0) quadrant is never computed; causal masking adds
  -1e30*max(0,s-t) to the two diagonal 128x128 blocks via an extra matmul
  into the score psum.
- softmax denominators come from ones-matmuls over two heads at a time (which
  also broadcasts them across partitions); normalization and the residual adds
  run on the otherwise-idle Pool (gpsimd) engine.
- the output projection packs head pairs on partitions so its contraction uses
  the full K=128 array.
- stages are software-pipelined across batch pairs in emission order
  C(p), B(p+1), D(p), A(p+2), E(p) so LN chains for pair p+2 overlap
  attention/FFN of pair p.

The v1 (float32r) builder is kept as a fallback for nonzero biases.
"""

import sys

if "/opt/trn_rl_repo" not in sys.path:
    sys.path.insert(0, "/opt/trn_rl_repo")

import numpy as np

import concourse.bass as bass
import concourse.mybir as mybir
import concourse.tile as tile
from concourse import bacc

# All ACT functions used here (Exp, Ln, Relu, Identity, Copy) live in the
# 'natural_log_exp_and_others' table set. Blank the other sets (preserving
# dict order, which defines act_func_set_id) so the table-load fixpoint
# settles on a single ACT_TABLE_LOAD instead of thrashing sets per batch.
_KEEP_ACT_SET = "natural_log_exp_and_others"
_orig_get_act_tables = bacc.get_activation_tables


def _one_set_tables(arch):
    t = _orig_get_act_tables(arch)
    assert _KEEP_ACT_SET in t
    return {k: (v if k == _KEEP_ACT_SET else set()) for k, v in t.items()}


bacc.get_activation_tables = _one_set_tables

F32 = mybir.dt.float32
F32R = mybir.dt.float32r
BF16 = mybir.dt.bfloat16
AF = mybir.ActivationFunctionType
ALU = mybir.AluOpType

B, T, C, H, D = 128, 256, 384, 6, 64
NCORES = 8
BL = B // NCORES          # batches per core
F = 4 * C                 # 1536
P = 128
TCH = T // P              # 2 token chunks
CCH = C // P              # 3 channel chunks
FCH = F // P              # 12 ffn chunks
HP = H // 2               # head pairs
HD = H * D                # 384
SCALE = float(C) ** -0.5  # reference scales by full model dim
EPS = 1e-5


# --------------------------------------------------------------------------
# v2 builder: fp8 DoubleRow GEMMs, split-diagonal attention, pool offload,
# software pipeline.
#
# Scaling scheme: fp8 weights are stored x16 (lifting sigma~0.02 weights out
# of the e4m3 subnormal range). Activations carried unscaled except:
#   q,k carry x16 -> scores carry x256 -> folded into the exp scale
#   v carries x16 -> op/osb carry x16; wo2 is x16 -> ap carries x256
#     -> xnew = ap/256 + x via scalar_tensor_tensor
#   hsm carries x16 (relu of x16 psum), w2 x16 -> fps carry x256
#     -> y = fps/256 + xnew
# --------------------------------------------------------------------------

FP8 = mybir.dt.float8e4
WSC = 16.0                 # fp8 weight prescale
INV2 = 1.0 / (WSC * WSC)


def build_program_v2(bl=BL, repeat=1, ps1_bufs=4, debug_dump=False):
    assert bl % 2 == 0
    NP = bl // 2

    nc = bacc.Bacc("TRN2", target_bir_lowering=False, debug=False,
                   num_devices=NCORES)

    x_d = nc.dram_tensor("x", [bl, T, C], BF16, kind="ExternalInput")
    wq_d = nc.dram_tensor("wq", [P, CCH, HD], FP8, kind="ExternalInput")
    wk_d = nc.dram_tensor("wk", [P, CCH, HD], FP8, kind="ExternalInput")
    wv_d = nc.dram_tensor("wv", [P, CCH, HD], FP8, kind="ExternalInput")
    wo_d = nc.dram_tensor("wo2", [P, HP, C], FP8, kind="ExternalInput")
    w1_d = nc.dram_tensor("w1", [P, CCH, F], FP8, kind="ExternalInput")
    w2_d = nc.dram_tensor("w2", [P, FCH, C], FP8, kind="ExternalInput")
    id_d = nc.dram_tensor("identb", [P, P], BF16, kind="ExternalInput")
    i2_d = nc.dram_tensor("id256", [P, P], BF16, kind="ExternalInput")
    i6_d = nc.dram_tensor("id16i", [P, P], F32R, kind="ExternalInput")
    on_d = nc.dram_tensor("ones8", [P, P], FP8, kind="ExternalInput")
    tl_d = nc.dram_tensor("trilmb", [P, P], BF16, kind="ExternalInput")
    ng_d = nc.dram_tensor("negmb", [P, TCH, T], BF16, kind="ExternalInput")
    y_d = nc.dram_tensor("y", [bl, T, C], F32, kind="ExternalOutput")
    dbg = {}
    if debug_dump:
        dbg["xnT2"] = nc.dram_tensor("d_xnT2", [P, CCH, 2, T], F32,
                                     kind="ExternalOutput")
        dbg["qsb2"] = nc.dram_tensor("d_qsb2", [P, CCH, 2, T], F32,
                                     kind="ExternalOutput")
        dbg["ksb2"] = nc.dram_tensor("d_ksb2", [P, CCH, 2, T], F32,
                                     kind="ExternalOutput")
        dbg["vsb"] = nc.dram_tensor("d_vsb", [P, TCH, HD], F32,
                                    kind="ExternalOutput")
        dbg["e"] = nc.dram_tensor("d_e", [P, H, 2, T], F32,
                                  kind="ExternalOutput")
        dbg["rbc"] = nc.dram_tensor("d_rbc", [P, H, T], F32,
                                    kind="ExternalOutput")
        dbg["osb"] = nc.dram_tensor("d_osb", [P, HP, T], F32,
                                    kind="ExternalOutput")
        dbg["xnew"] = nc.dram_tensor("d_xnew", [P, TCH, C], F32,
                                     kind="ExternalOutput")
        dbg["vsb1"] = nc.dram_tensor("d_vsb1", [P, TCH, HD], F32,
                                     kind="ExternalOutput")
        dbg["e1"] = nc.dram_tensor("d_e1", [P, H, 2, T], F32,
                                   kind="ExternalOutput")
        dbg["rbc1"] = nc.dram_tensor("d_rbc1", [P, H, T], F32,
                                     kind="ExternalOutput")
        dbg["osb1"] = nc.dram_tensor("d_osb1", [P, HP, T], F32,
                                     kind="ExternalOutput")
        dbg["xnew1"] = nc.dram_tensor("d_xnew1", [P, TCH, C], F32,
                                      kind="ExternalOutput")
        dbg["xn2T2"] = nc.dram_tensor("d_xn2T2", [P, CCH, 2, T], F32,
                                      kind="ExternalOutput")
        dbg["hsm"] = nc.dram_tensor("d_hsm", [P, 2, 2, T], F32,
                                    kind="ExternalOutput")

    with tile.TileContext(nc) as tc:
        with (
            tc.tile_pool(name="wpool", bufs=1) as wp,
            tc.tile_pool(name="work", bufs=2) as wk,
            tc.tile_pool(name="ps1", bufs=ps1_bufs, space="PSUM") as ps1,
            tc.tile_pool(name="psb", bufs=1, space="PSUM") as psb,
        ):
            wq = wp.tile([P, CCH, HD], FP8)
            wkk = wp.tile([P, CCH, HD], FP8)
            wv = wp.tile([P, CCH, HD], FP8)
            wo2 = wp.tile([P, HP, C], FP8)
            w1 = wp.tile([P, CCH, F], FP8)
            w2 = wp.tile([P, FCH, C], FP8)
            identb = wp.tile([P, P], BF16)
            id256 = wp.tile([P, P], BF16)
            id16i = wp.tile([P, P], F32R)
            ones8 = wp.tile([P, P], FP8)
            trilmb = wp.tile([P, P], BF16)
            negmb = wp.tile([P, TCH, T], BF16)
            epsb = wp.tile([P, 1], F32)
            nc.gpsimd.memset(epsb[:], EPS)
            # identb first: the first PE work (LN1 transposes of pair 0)
            # needs only it and the x DMA, which stage_A(0) emits next.
            nc.sync.dma_start(identb[:], id_d[:])

            def load_weights():
                nc.sync.dma_start(wq[:], wq_d[:])
                nc.sync.dma_start(wkk[:], wk_d[:])
                nc.sync.dma_start(wv[:], wv_d[:])
                nc.sync.dma_start(trilmb[:], tl_d[:])
                nc.sync.dma_start(negmb[:], ng_d[:])
                nc.sync.dma_start(ones8[:], on_d[:])
                nc.sync.dma_start(wo2[:], wo_d[:])
                nc.sync.dma_start(id256[:], i2_d[:])
                nc.sync.dma_start(w1[:], w1_d[:])
                nc.sync.dma_start(w2[:], w2_d[:])
                nc.sync.dma_start(id16i[:], i6_d[:])

            DRm = mybir.MatmulPerfMode.DoubleRow

            def layer_norm_T(src, dstT, i, tagpfx, evac_act,
                             norm_eng="pool"):
                """src [P,TCH,C] (bf16 or f32r); (src-mu)*rstd -> bf16 ->
                transposed -> dstT[:, :, i, :] in fp8."""
                st6 = wk.tile([P, TCH, 6], F32, tag=f"{tagpfx}st6_{i}")
                mv = wk.tile([P, TCH, 2], F32, tag=f"{tagpfx}mv_{i}")
                rstd = wk.tile([P, TCH], F32, tag=f"{tagpfx}rstd_{i}")
                for tch in range(TCH):
                    nc.vector.bn_stats(st6[:, tch, :], src[:, tch, :])
                    nc.vector.bn_aggr(mv[:, tch, :], st6[:, tch, :])
                # rstd = exp(-0.5 * ln(var + eps))
                nc.scalar.activation(rstd[:], mv[:, :, 1], AF.Ln, bias=epsb[:])
                nc.scalar.activation(rstd[:], rstd[:], AF.Exp, scale=-0.5)
                xn = wk.tile([P, TCH, C], BF16, tag=f"{tagpfx}xn_{i}", bufs=1)
                neng = nc.gpsimd if norm_eng == "pool" else nc.vector
                for tch in range(TCH):
                    neng.tensor_scalar(
                        xn[:, tch, :], src[:, tch, :],
                        mv[:, tch, 0:1], rstd[:, tch:tch + 1],
                        ALU.subtract, ALU.mult,
                    )
                trAB = ps1.tile([P, CCH, T], BF16, tag="ps1", name="trAB")
                for tch in range(TCH):
                    for cc in range(CCH):
                        nc.tensor.transpose(
                            trAB[:, cc, tch * P:(tch + 1) * P],
                            xn[:, tch, cc * P:(cc + 1) * P], identb[:])
                if evac_act:
                    nc.scalar.copy(dstT[:, :, i, :], trAB[:])
                else:
                    nc.vector.tensor_copy(dstT[:, :, i, :], trAB[:])

            state = {}

            def dump(name, ap):
                if not debug_dump or name not in dbg:
                    return
                dt = wk.tile(list(ap.shape), F32, tag=f"dump_{name}", bufs=1)
                nc.vector.tensor_copy(dt[:], ap)
                nc.sync.dma_start(dbg.pop(name)[:], dt[:])

            def stage_A(p):
                """x DMA + LN1 + transposes -> xnT2(p)."""
                bp = (2 * p, 2 * p + 1)
                xts = []
                xnT2 = wk.tile([P, CCH, 2, T], FP8, tag="xnT2")
                for i, b in enumerate(bp):
                    xt = wk.tile([P, TCH, C], BF16, tag=f"xt{i}")
                    nc.sync.dma_start(
                        xt[:], x_d[b].rearrange("(tc p) c -> p tc c", p=P))
                    xts.append(xt)
                    layer_norm_T(xt, xnT2, i, "a", evac_act=(i == 0))
                if p == 0:
                    dump("xnT2", xnT2[:, :, :, :])
                state[p] = {"xts": xts, "xnT2": xnT2}

            def stage_B(p):
                """QKV projections from xnT2(p). q,k,v carry x16."""
                st = state[p]
                xnT2 = st["xnT2"]
                qsb2 = wk.tile([P, CCH, 2, T], BF16, tag="qsb2")
                ksb2 = wk.tile([P, CCH, 2, T], BF16, tag="ksb2")
                for wmat, dst, eng in ((wq, qsb2, "act"), (wkk, ksb2, "dve")):
                    for mc in range(CCH):
                        pp = ps1.tile([P, 2, T], F32, tag="ps1", name="pp")
                        nc.tensor.matmul(
                            pp[:, :, :],
                            wmat[:, 0:2, mc * P:(mc + 1) * P],
                            xnT2[:, 0:2, :, :],
                            start=True, stop=False, perf_mode=DRm)
                        nc.tensor.matmul(
                            pp[:, :, :],
                            wmat[:, 2, mc * P:(mc + 1) * P],
                            xnT2[:, 2, :, :],
                            start=False, stop=True)
                        if eng == "act":
                            nc.scalar.copy(dst[:, mc, :, :], pp[:])
                        else:
                            nc.vector.tensor_copy(dst[:, mc, :, :], pp[:])
                vsbs = []
                for i in range(2):
                    vsb = wk.tile([P, TCH, HD], FP8, tag=f"vsb{i}")
                    vsbs.append(vsb)
                    for sc in range(TCH):
                        vp = ps1.tile([P, HD], F32, tag="ps1", name="vp")
                        nc.tensor.matmul(
                            vp[:, :],
                            xnT2[:, 0:2, i, sc * P:(sc + 1) * P],
                            wv[:, 0:2, :],
                            start=True, stop=False, perf_mode=DRm)
                        nc.tensor.matmul(
                            vp[:, :],
                            xnT2[:, 2, i, sc * P:(sc + 1) * P],
                            wv[:, 2, :],
                            start=False, stop=True)
                        if sc == 0:
                            nc.scalar.copy(vsb[:, sc, :], vp[:])
                        else:
                            nc.vector.tensor_copy(vsb[:, sc, :], vp[:])
                if p == 0:
                    dump("qsb2", qsb2[:, :, :, :])
                    dump("ksb2", ksb2[:, :, :, :])
                    dump("vsb", vsbs[0][:, :, :])
                    dump("vsb1", vsbs[1][:, :, :])
                st["qsb2"] = qsb2
                st["ksb2"] = ksb2
                st["vsbs"] = vsbs

            def stage_C(p):
                """Attention + out-proj + residual -> xnews(p) (carrying
                x256)."""
                st = state[p]
                qsb2, ksb2, vsbs, xts = (st["qsb2"], st["ksb2"], st["vsbs"],
                                         st["xts"])
                osbs = []
                for i in range(2):
                    # e [P, H, sc, T] fp8; masked entries exp to exact 0
                    e_all = wk.tile([P, H, 2, T], FP8, tag=f"e{i}")
                    rbc = wk.tile([P, H, T], F32, tag=f"rbc{i}")
                    state[p][f"e{i}"] = e_all
                    state[p][f"rbc{i}"] = rbc
                    osb = wk.tile([P, HP, T], FP8, tag=f"osb{i}", bufs=1)
                    osbs.append(osb)
                    for j in range(HP):  # head pair
                        for j2 in range(2):
                            h = 2 * j + j2
                            hc, ho = h // 2, 64 * (h % 2)
                            kap = ksb2[ho:ho + D, hc, i, :]
                            qap = qsb2[ho:ho + D, hc, i, :]
                            sp = ps1.tile([P, 2, T], F32, tag="ps1",
                                          name="sp")
                            for sc in range(TCH):
                                nc.tensor.matmul(
                                    sp[:, sc, :],
                                    kap[:, sc * P:(sc + 1) * P], qap[:, :],
                                    start=True, stop=False,
                                    skip_group_check=True)
                            nc.tensor.matmul(
                                sp[:, :, :], trilmb[:], negmb[:, :, :],
                                start=False, stop=True, skip_group_check=True)
                            # q,k carry x16 each -> scores carry x256
                            nc.scalar.activation(
                                e_all[:, h, :, :], sp[:], AF.Exp,
                                scale=SCALE * INV2)
                        # denominators for the pair (broadcast over
                        # partitions); skip the all-masked quadrant
                        dp = ps1.tile([P, 2, T], F32, tag="ps1", name="dp")
                        nc.tensor.matmul(
                            dp[:, :, :], ones8[:],
                            e_all[:, 2 * j:2 * j + 2, 0, :],
                            start=True, stop=False)
                        nc.tensor.matmul(
                            dp[:, :, 128:256], ones8[:],
                            e_all[:, 2 * j:2 * j + 2, 1, 128:256],
                            start=False, stop=True, skip_group_check=True)
                        nc.vector.reciprocal_approx_fast(
                            rbc[:, 2 * j:2 * j + 2, :], dp[:])
                        # attn @ v, both s-chunks in one DoubleRow per head
                        op2 = ps1.tile([D, 2, T], F32, tag="ps1", name="op2")
                        for j2 in range(2):
                            h = 2 * j + j2
                            nc.tensor.matmul(
                                op2[:, j2, :],
                                vsbs[i][:, :, h * D:(h + 1) * D],
                                e_all[:, h, :, :],
                                start=True, stop=True, perf_mode=DRm,
                                skip_group_check=True)
                        for j2 in range(2):
                            h = 2 * j + j2
                            nc.vector.tensor_tensor(
                                osb[j2 * D:(j2 + 1) * D, j, :],
                                op2[0:D, j2, :],
                                rbc[j2 * D:(j2 + 1) * D, h, :], ALU.mult)

                if p == 0:
                    dump("e", state[p]["e0"][:, :, :, :])
                    dump("rbc", state[p]["rbc0"][:, :, :])
                    dump("osb", osbs[0][:, :, :])
                    dump("e1", state[p]["e1"][:, :, :, :])
                    dump("rbc1", state[p]["rbc1"][:, :, :])
                    dump("osb1", osbs[1][:, :, :])
                # out-proj (head pairs packed on partitions) + residual via
                # scaled-identity matmul; evacuated as one copy per batch
                xnews = []
                for i in range(2):
                    # 512-wide slots keep each matmul output inside one
                    # 2KB PSUM bank
                    ap2 = psb.tile([P, 4, 512], F32, tag="big", name="ap2")
                    for tcc in range(TCH):
                        nc.tensor.matmul(
                            ap2[:, tcc, 0:C],
                            osbs[i][:, 0:2, tcc * P:(tcc + 1) * P],
                            wo2[:, 0:2, :],
                            start=True, stop=False, perf_mode=DRm,
                            skip_group_check=True)
                        nc.tensor.matmul(
                            ap2[:, tcc, 0:C],
                            osbs[i][:, 2, tcc * P:(tcc + 1) * P],
                            wo2[:, 2, :],
                            start=False, stop=False, skip_group_check=True)
                        # + 256*x residual (osb x16 * wo2 x16 = x256 domain)
                        nc.tensor.matmul(
                            ap2[:, tcc, 0:C], id256[:], xts[i][:, tcc, :],
                            start=False, stop=True, skip_group_check=True)
                    xnew = wk.tile([P, TCH, C], F32R, tag=f"xnew{i}")
                    xnews.append(xnew)
                    if i == 0:
                        nc.scalar.copy(xnew[:, :, :], ap2[:, 0:TCH, 0:C])
                    else:
                        nc.vector.tensor_copy(xnew[:, :, :],
                                              ap2[:, 0:TCH, 0:C])
                if p == 0:
                    dump("xnew", xnews[0][:, :, :])
                    dump("xnew1", xnews[1][:, :, :])
                st["xnews"] = xnews

            def stage_D(p):
                """LN2 + transposes -> xn2T2(p). LN is scale-invariant, so
                the x256 carried by xnew drops out here."""
                st = state[p]
                xn2T2 = wk.tile([P, CCH, 2, T], FP8, tag="xn2T2")
                for i in range(2):
                    layer_norm_T(st["xnews"][i], xn2T2, i, "d",
                                 evac_act=(i == 1))
                if p == 0:
                    dump("xn2T2", xn2T2[:, :, :, :])
                st["xn2T2"] = xn2T2

            def stage_E(p):
                """FFN + residual + store. hsm true scale, w2 x16."""
                st = state[p]
                bp = (2 * p, 2 * p + 1)
                xn2T2 = st["xn2T2"]
                fpsb = psb.tile([P, 4, 512], F32, tag="big", name="fpsb")

                def fslot(i, tcc):
                    return fpsb[:, 2 * i + tcc, 0:C]

                for mp in range(FCH // 2):
                    # hsm [P, mo-sub, batch, T]: mo-subtile dim is the
                    # DoubleRow k-pair for FFN2
                    hsm = wk.tile([P, 2, 2, T], FP8, tag="hsm", bufs=3)
                    for moj in range(2):
                        mo = 2 * mp + moj
                        hp = ps1.tile([P, 2, T], F32, tag="ps1", name="hp")
                        nc.tensor.matmul(
                            hp[:, :, :],
                            w1[:, 0:2, mo * P:(mo + 1) * P],
                            xn2T2[:, 0:2, :, :],
                            start=True, stop=False, perf_mode=DRm)
                        nc.tensor.matmul(
                            hp[:, :, :],
                            w1[:, 2, mo * P:(mo + 1) * P],
                            xn2T2[:, 2, :, :],
                            start=False, stop=True)
                        # hp carries x16 -> scale back to true h
                        if moj == 0:
                            nc.scalar.activation(
                                hsm[:, moj, :, :], hp[:], AF.Relu,
                                scale=1.0 / WSC)
                        else:
                            nc.vector.tensor_scalar(
                                hsm[:, moj, :, :], hp[:], 1.0 / WSC, 0.0,
                                ALU.mult, ALU.max)
                    if p == 0 and mp == 0:
                        dump("hsm", hsm[:, :, :, :])
                    for i in range(2):
                        for tcc in range(TCH):
                            nc.tensor.matmul(
                                fslot(i, tcc),
                                hsm[:, :, i, tcc * P:(tcc + 1) * P],
                                w2[:, 2 * mp:2 * mp + 2, :],
                                start=(mp == 0), stop=False,
                                perf_mode=DRm, skip_group_check=True)
                # fps carries x16 (h * 16w2); add xnew256/16 so one /16
                # evacuation yields y
                for i in range(2):
                    for tcc in range(TCH):
                        nc.tensor.matmul(
                            fslot(i, tcc), id16i[:],
                            st["xnews"][i][:, tcc, :],
                            start=False, stop=True, skip_group_check=True)
                yout = wk.tile([P, 4, C], F32, tag="yout")
                nc.scalar.activation(yout[:], fpsb[:, :, 0:C], AF.Copy,
                                     scale=1.0 / WSC)
                for i, b in enumerate(bp):
                    nc.sync.dma_start(
                        y_d[b].rearrange("(tc p) c -> p tc c", p=P),
                        yout[:, 2 * i:2 * i + 2, :])

            def body(emit_weights):
                state.clear()
                stage_A(0)
                if emit_weights:
                    load_weights()
                stage_B(0)
                if NP > 1:
                    stage_A(1)
                for p in range(NP):
                    stage_C(p)
                    if p + 1 < NP:
                        stage_B(p + 1)
                    stage_D(p)
                    if p + 2 < NP:
                        stage_A(p + 2)
                    stage_E(p)
                    if p - 1 in state:
                        del state[p - 1]

            if repeat > 1:
                load_weights()
                with tc.For_i(0, repeat, 1):
                    body(emit_weights=False)
            else:
                body(emit_weights=True)

    nc.compile()
    return nc


def prep_weights_v2(Wq, Wk, Wv, Wo, bo, W1, b1, W2, b2, g1, be1, g2, be2):
    import ml_dtypes
    bf = ml_dtypes.bfloat16
    f8 = ml_dtypes.float8_e4m3
    f32 = np.float32

    def kchunk(w, kdim):  # [K, M] -> [P, K//P, M], x16 in fp8
        m = w.shape[1]
        return np.ascontiguousarray(
            (WSC * w).reshape(kdim // P, P, m).transpose(1, 0, 2)).astype(f8)

    Wq2 = Wq.transpose(1, 0, 2).reshape(C, HD)
    Wk2 = Wk.transpose(1, 0, 2).reshape(C, HD)
    Wv2 = Wv.transpose(1, 0, 2).reshape(C, HD)
    WoR = Wo.reshape(H, D, C)
    wo2 = np.zeros((P, HP, C), dtype=f32)
    for h in range(H):
        wo2[64 * (h % 2):64 * (h % 2) + 64, h // 2, :] = WoR[h]
    out = {
        "wq": kchunk(g1[:, None] * Wq2, C),
        "wk": kchunk(g1[:, None] * Wk2, C),
        "wv": kchunk(g1[:, None] * Wv2, C),
        "wo2": (WSC * wo2).astype(f8),
        "w1": kchunk(g2[:, None] * W1, C),
        "w2": kchunk(W2, F),
        "identb": np.eye(P, dtype=f32).astype(bf),
        "id256": (256.0 * np.eye(P, dtype=f32)).astype(bf),
        "id16i": (np.eye(P, dtype=f32) / 16.0),
        "ones8": np.ones((P, P), dtype=f32).astype(f8),
        "trilmb": np.tril(np.ones((P, P), dtype=f32)).T.copy().astype(bf),
        "negmb": _make_negm().astype(bf),
    }
    return out


# --------------------------------------------------------------------------
# v1 builder (float32r, kept as fallback for nonzero biases)
# --------------------------------------------------------------------------

def build_program(bl=BL, flags=frozenset(), repeat=1,
                  tr_split=False, tr_bufs=1, ps1_bufs=6, wk_bufs=2):
    """Per-core Bass program. `flags` lists nonzero bias terms
    ('qb','kb','vb','bo','b1','b2'). `repeat` wraps the whole computation in
    a hardware loop (benchmarking only)."""
    assert bl % 2 == 0
    use_qb = "qb" in flags
    use_kb = "kb" in flags
    use_vb = "vb" in flags
    use_bo = "bo" in flags
    use_b1 = "b1" in flags
    use_b2 = "b2" in flags

    nc = bacc.Bacc("TRN2", target_bir_lowering=False, debug=False,
                   num_devices=NCORES)

    x_d = nc.dram_tensor("x", [bl, T, C], F32, kind="ExternalInput")
    wq_d = nc.dram_tensor("wq", [P, CCH, HD], F32R, kind="ExternalInput")
    wk_d = nc.dram_tensor("wk", [P, CCH, HD], F32R, kind="ExternalInput")
    wv_d = nc.dram_tensor("wv", [P, CCH, HD], F32R, kind="ExternalInput")
    qb_d = nc.dram_tensor("qb", [P, CCH], F32, kind="ExternalInput")
    kb_d = nc.dram_tensor("kb", [P, CCH], F32, kind="ExternalInput")
    vb_d = nc.dram_tensor("vb", [1, HD], F32R, kind="ExternalInput")
    wo_d = nc.dram_tensor("wo", [D, H, C], F32R, kind="ExternalInput")
    bo_d = nc.dram_tensor("bo", [1, C], F32R, kind="ExternalInput")
    w1_d = nc.dram_tensor("w1", [P, CCH, F], F32R, kind="ExternalInput")
    b1_d = nc.dram_tensor("b1c", [P, FCH], F32, kind="ExternalInput")
    w2_d = nc.dram_tensor("w2", [P, FCH, C], F32R, kind="ExternalInput")
    b2_d = nc.dram_tensor("b2", [1, C], F32R, kind="ExternalInput")
    id_d = nc.dram_tensor("ident", [P, P], F32R, kind="ExternalInput")
    on_d = nc.dram_tensor("onesm", [P, P], F32R, kind="ExternalInput")
    tl_d = nc.dram_tensor("trilm", [P, P], F32R, kind="ExternalInput")
    ng_d = nc.dram_tensor("negm", [P, TCH, T], F32R, kind="ExternalInput")
    y_d = nc.dram_tensor("y", [bl, T, C], F32, kind="ExternalOutput")
    dbg = {}
    if debug_dump:
        dbg["xnT2"] = nc.dram_tensor("d_xnT2", [P, CCH, 2, T], F32,
                                     kind="ExternalOutput")
        dbg["qsb2"] = nc.dram_tensor("d_qsb2", [P, CCH, 2, T], F32,
                                     kind="ExternalOutput")
        dbg["ksb2"] = nc.dram_tensor("d_ksb2", [P, CCH, 2, T], F32,
                                     kind="ExternalOutput")
        dbg["vsb"] = nc.dram_tensor("d_vsb", [P, TCH, HD], F32,
                                    kind="ExternalOutput")
        dbg["e"] = nc.dram_tensor("d_e", [P, H, 2, T], F32,
                                  kind="ExternalOutput")
        dbg["rbc"] = nc.dram_tensor("d_rbc", [P, H, T], F32,
                                    kind="ExternalOutput")
        dbg["osb"] = nc.dram_tensor("d_osb", [P, HP, T], F32,
                                    kind="ExternalOutput")
        dbg["xnew"] = nc.dram_tensor("d_xnew", [P, TCH, C], F32,
                                     kind="ExternalOutput")
        dbg["vsb1"] = nc.dram_tensor("d_vsb1", [P, TCH, HD], F32,
                                     kind="ExternalOutput")
        dbg["e1"] = nc.dram_tensor("d_e1", [P, H, 2, T], F32,
                                   kind="ExternalOutput")
        dbg["rbc1"] = nc.dram_tensor("d_rbc1", [P, H, T], F32,
                                     kind="ExternalOutput")
        dbg["osb1"] = nc.dram_tensor("d_osb1", [P, HP, T], F32,
                                     kind="ExternalOutput")
        dbg["xnew1"] = nc.dram_tensor("d_xnew1", [P, TCH, C], F32,
                                      kind="ExternalOutput")
        dbg["xn2T2"] = nc.dram_tensor("d_xn2T2", [P, CCH, 2, T], F32,
                                      kind="ExternalOutput")
        dbg["hsm"] = nc.dram_tensor("d_hsm", [P, 2, 2, T], F32,
                                    kind="ExternalOutput")

    with tile.TileContext(nc) as tc:
        import contextlib
        with (
            tc.tile_pool(name="wpool", bufs=1) as wp,
            tc.tile_pool(name="work", bufs=wk_bufs) as wk_pool,
            tc.tile_pool(name="big", bufs=1) as bigp,
            (contextlib.nullcontext(None) if tr_split else
             tc.tile_pool(name="ps_tr", bufs=tr_bufs, space="PSUM")) as pstr,
            tc.tile_pool(name="ps_one", bufs=ps1_bufs, space="PSUM") as ps1,
        ):
            # ---- load weights/constants once ----
            wq = wp.tile([P, CCH, HD], F32R)
            wkk = wp.tile([P, CCH, HD], F32R)
            wv = wp.tile([P, CCH, HD], F32R)
            wo = wp.tile([D, H, C], F32R)
            w1 = wp.tile([P, CCH, F], F32R)
            w2 = wp.tile([P, FCH, C], F32R)
            ident = wp.tile([P, P], F32R)
            ones_t = wp.tile([P, P], F32R)
            trilm = wp.tile([P, P], F32R)
            negm = wp.tile([P, TCH, T], F32R)
            epsb = wp.tile([P, 1], F32)
            nc.gpsimd.memset(epsb[:], EPS)
            nc.sync.dma_start(wq[:], wq_d[:])
            nc.sync.dma_start(wkk[:], wk_d[:])
            nc.sync.dma_start(wv[:], wv_d[:])
            nc.sync.dma_start(wo[:], wo_d[:])
            nc.sync.dma_start(w1[:], w1_d[:])
            nc.sync.dma_start(w2[:], w2_d[:])
            nc.sync.dma_start(ident[:], id_d[:])
            nc.sync.dma_start(ones_t[:], on_d[:])
            nc.sync.dma_start(trilm[:], tl_d[:])
            nc.sync.dma_start(negm[:], ng_d[:])
            qb = kb = vb = bo = b1c = b2 = None
            if use_qb:
                qb = wp.tile([P, CCH], F32)
                nc.sync.dma_start(qb[:], qb_d[:])
            if use_kb:
                kb = wp.tile([P, CCH], F32)
                nc.sync.dma_start(kb[:], kb_d[:])
            if use_vb:
                vb = wp.tile([1, HD], F32R)
                nc.sync.dma_start(vb[:], vb_d[:])
            if use_bo:
                bo = wp.tile([1, C], F32R)
                nc.sync.dma_start(bo[:], bo_d[:])
            if use_b1:
                b1c = wp.tile([P, FCH], F32)
                nc.sync.dma_start(b1c[:], b1_d[:])
            if use_b2:
                b2 = wp.tile([1, C], F32R)
                nc.sync.dma_start(b2[:], b2_d[:])

            def layer_norm_T(src, dstT, i, evac_act):
                st6 = wk_pool.tile([P, TCH, 6], F32, tag=f"st6_{i}")
                mv = wk_pool.tile([P, TCH, 2], F32, tag=f"mv_{i}")
                rstd = wk_pool.tile([P, TCH], F32, tag=f"rstd_{i}")
                for tch in range(TCH):
                    nc.vector.bn_stats(st6[:, tch, :], src[:, tch, :])
                    nc.vector.bn_aggr(mv[:, tch, :], st6[:, tch, :])
                nc.scalar.activation(rstd[:], mv[:, :, 1], AF.Ln, bias=epsb[:])
                nc.scalar.activation(rstd[:], rstd[:], AF.Exp, scale=-0.5)
                xn = wk_pool.tile([P, TCH, C], F32R, tag=f"xn_{i}", bufs=1)
                for tch in range(TCH):
                    nc.vector.tensor_scalar(
                        xn[:, tch, :], src[:, tch, :],
                        mv[:, tch, 0:1], rstd[:, tch:tch + 1],
                        ALU.subtract, ALU.mult,
                    )
                trA = ps1.tile([P, 2, T], F32R, tag="ps1", name="trA")
                trB = ps1.tile([P, T], F32R, tag="ps1", name="trB")

                def _trdst(cc):
                    return trB if cc == 2 else trA[:, cc, :]
                for tch in range(TCH):
                    for cc in range(CCH):
                        nc.tensor.transpose(
                            _trdst(cc)[:, tch * P:(tch + 1) * P],
                            xn[:, tch, cc * P:(cc + 1) * P],
                            ident[:],
                        )
                if evac_act:
                    nc.scalar.copy(dstT[:, 0:2, i, :], trA[:])
                    nc.scalar.copy(dstT[:, 2, i, :], trB[:])
                else:
                    nc.vector.tensor_copy(dstT[:, 0:2, i, :], trA[:])
                    nc.vector.tensor_copy(dstT[:, 2, i, :], trB[:])

            def body():
                for pb in range(bl // 2):
                    bp = (2 * pb, 2 * pb + 1)
                    xts = []
                    xnT2 = wk_pool.tile([P, CCH, 2, T], F32R, tag="xnT2")
                    for i, b in enumerate(bp):
                        xt = wk_pool.tile([P, TCH, C], F32, tag=f"xt{i}")
                        nc.sync.dma_start(
                            xt[:], x_d[b].rearrange("(tc p) c -> p tc c", p=P))
                        xts.append(xt)
                        layer_norm_T(xt, xnT2, i, evac_act=(i == 0))

                    qsb2 = wk_pool.tile([P, CCH, 2, T], F32R, tag="qsb2")
                    ksb2 = wk_pool.tile([P, CCH, 2, T], F32R, tag="ksb2", bufs=1)
                    for wmat, bias_t, use_b, dst, eng in (
                        (wq, qb, use_qb, qsb2, "act"),
                        (wkk, kb, use_kb, ksb2, "dve"),
                    ):
                        for mc in range(CCH):
                            pp = ps1.tile([P, 2, T], F32, tag="ps1")
                            for kc in range(CCH):
                                nc.tensor.matmul(
                                    pp[:, :, :],
                                    wmat[:, kc, mc * P:(mc + 1) * P],
                                    xnT2[:, kc, :, :],
                                    start=(kc == 0), stop=(kc == CCH - 1),
                                )
                            if use_b:
                                nc.scalar.activation(
                                    dst[:, mc, :, :], pp[:], AF.Identity,
                                    bias=bias_t[:, mc:mc + 1])
                            elif eng == "act":
                                nc.scalar.copy(dst[:, mc, :, :], pp[:])
                            else:
                                nc.vector.tensor_copy(dst[:, mc, :, :], pp[:])

                    vsbs = []
                    for i in range(2):
                        vsb = wk_pool.tile([P, TCH, HD], F32R, tag=f"vsb{i}")
                        vsbs.append(vsb)
                        for sc in range(TCH):
                            vp = ps1.tile([P, HD], F32, tag="ps1")
                            for kc in range(CCH):
                                nc.tensor.matmul(
                                    vp[:, :],
                                    xnT2[:, kc, i, sc * P:(sc + 1) * P],
                                    wv[:, kc, :],
                                    start=(kc == 0),
                                    stop=(kc == CCH - 1 and not use_vb),
                                )
                            if use_vb:
                                nc.tensor.matmul(
                                    vp[:, :], ones_t[0:1, :], vb[0:1, :],
                                    start=False, stop=True)
                            if sc == 0:
                                nc.scalar.copy(vsb[:, sc, :], vp[:])
                            else:
                                nc.vector.tensor_copy(vsb[:, sc, :], vp[:])

                    xnews = []
                    for i, b in enumerate(bp):
                        e_all = bigp.tile([P, TCH, H, T], F32R, tag="e_all")
                        rbc = bigp.tile([P, H, T], F32, tag="rbc")
                        osb = wk_pool.tile([64, H, T], F32R, tag="osb", bufs=1)
                        for h in range(H):
                            hc, ho = h // 2, 64 * (h % 2)
                            sp = ps1.tile([P, TCH, T], F32, tag="ps1")
                            for sc in range(TCH):
                                nc.tensor.matmul(
                                    sp[:, sc, :],
                                    ksb2[ho:ho + D, hc, i, sc * P:(sc + 1) * P],
                                    qsb2[ho:ho + D, hc, i, :],
                                    start=True, stop=False,
                                )
                                nc.tensor.matmul(
                                    sp[:, sc, :], trilm[:, :], negm[:, sc, :],
                                    start=False, stop=True,
                                )
                            nc.scalar.activation(
                                e_all[:, :, h, :], sp[:], AF.Exp, scale=SCALE)
                        for pc in range(H // 2):
                            dp = ps1.tile([P, 2, T], F32, tag="ps1")
                            ops = []
                            for j in range(2):
                                h = 2 * pc + j
                                for sc in range(TCH):
                                    nc.tensor.matmul(
                                        dp[:, j, :], ones_t[:],
                                        e_all[:, sc, h, :],
                                        start=(sc == 0), stop=(sc == TCH - 1))
                                op_h = ps1.tile([D, T], F32, tag="ps1")
                                ops.append(op_h)
                                for sc in range(TCH):
                                    nc.tensor.matmul(
                                        op_h[:, :],
                                        vsbs[i][:, sc, h * D:(h + 1) * D],
                                        e_all[:, sc, h, :],
                                        start=(sc == 0), stop=(sc == TCH - 1))
                            nc.vector.reciprocal(
                                rbc[:, 2 * pc:2 * pc + 2, :], dp[:])
                            for j in range(2):
                                h = 2 * pc + j
                                nc.vector.tensor_tensor(
                                    osb[:, h, :], ops[j][:, :], rbc[0:D, h, :],
                                    ALU.mult,
                                )

                        xnew = wk_pool.tile([P, TCH, C], F32, tag=f"xnew{i}")
                        xnews.append(xnew)
                        for tcc in range(TCH):
                            ap_t = ps1.tile([P, C], F32, tag="ps1")
                            for h in range(H):
                                nc.tensor.matmul(
                                    ap_t[:, :],
                                    osb[:, h, tcc * P:(tcc + 1) * P],
                                    wo[:, h, :],
                                    start=(h == 0),
                                    stop=(h == H - 1 and not use_bo))
                            if use_bo:
                                nc.tensor.matmul(
                                    ap_t[:, :], ones_t[0:1, :], bo[0:1, :],
                                    start=False, stop=True)
                            nc.vector.tensor_tensor(
                                xnew[:, tcc, :], ap_t[:, :], xts[i][:, tcc, :],
                                ALU.add)

                    xn2T2 = wk_pool.tile([P, CCH, 2, T], F32R, tag="xn2T2")
                    for i in range(2):
                        layer_norm_T(xnews[i], xn2T2, i, evac_act=(i == 1))

                    fps = []
                    for j in range(4):
                        fp_j = ps1.tile([P, C], F32, tag="ps1", name=f"fp{j}")
                        fps.append(fp_j)
                    for mo in range(FCH):
                        hp = ps1.tile([P, 2, T], F32, tag="ps1")
                        for kc in range(CCH):
                            nc.tensor.matmul(
                                hp[:, :, :],
                                w1[:, kc, mo * P:(mo + 1) * P],
                                xn2T2[:, kc, :, :],
                                start=(kc == 0), stop=(kc == CCH - 1))
                        hsm = wk_pool.tile([P, 2, T], F32R, tag="hsm")
                        if use_b1:
                            nc.scalar.activation(
                                hsm[:], hp[:], AF.Relu, bias=b1c[:, mo:mo + 1])
                        else:
                            nc.scalar.activation(hsm[:], hp[:], AF.Relu)
                        for i in range(2):
                            for tcc in range(TCH):
                                nc.tensor.matmul(
                                    fps[2 * i + tcc][:, :],
                                    hsm[:, i, tcc * P:(tcc + 1) * P],
                                    w2[:, mo, :],
                                    start=(mo == 0),
                                    stop=(mo == FCH - 1 and not use_b2))

                    for i, b in enumerate(bp):
                        yout = wk_pool.tile([P, TCH, C], F32, tag=f"yout{i}")
                        for tcc in range(TCH):
                            fp = fps[2 * i + tcc]
                            if use_b2:
                                nc.tensor.matmul(
                                    fp[:, :], ones_t[0:1, :], b2[0:1, :],
                                    start=False, stop=True)
                            nc.vector.tensor_tensor(
                                yout[:, tcc, :], fp[:, :], xnews[i][:, tcc, :],
                                ALU.add)
                        nc.sync.dma_start(
                            y_d[b].rearrange("(tc p) c -> p tc c", p=P),
                            yout[:])

            if repeat > 1:
                with tc.For_i(0, repeat, 1):
                    body()
            else:
                body()

    nc.compile()
    return nc


def _make_negm():
    BIG = np.float32(1e30)
    f32 = np.float32
    m = np.zeros((P, TCH, T), dtype=f32)
    jgt = np.tril(np.ones((P, P), dtype=f32), -1)  # [j, t] = 1 iff j > t
    m[:, 0, 0:P] = -BIG * jgt
    m[:, 1, 0:P] = -BIG
    m[:, 1, P:2 * P] = -BIG * jgt
    return m


def prep_weights(Wq, Wk, Wv, Wo, bo, W1, b1, W2, b2, g1, be1, g2, be2):
    """Fold LN gamma/beta into projection weights; rearrange to SBUF layouts."""
    f32 = np.float32

    def kchunk(w, kdim):  # [K, M] -> [P, K//P, M]
        m = w.shape[1]
        return np.ascontiguousarray(
            w.reshape(kdim // P, P, m).transpose(1, 0, 2)).astype(f32)

    Wq2 = Wq.transpose(1, 0, 2).reshape(C, HD)
    Wk2 = Wk.transpose(1, 0, 2).reshape(C, HD)
    Wv2 = Wv.transpose(1, 0, 2).reshape(C, HD)
    out = {
        "wq": kchunk(g1[:, None] * Wq2, C),
        "wk": kchunk(g1[:, None] * Wk2, C),
        "wv": kchunk(g1[:, None] * Wv2, C),
        "wo": np.ascontiguousarray(
            Wo.reshape(H, D, C).transpose(1, 0, 2)).astype(f32),
        "w1": kchunk(g2[:, None] * W1, C),
        "w2": kchunk(W2, F),
        "ident": np.eye(P, dtype=f32),
        "onesm": np.ones((P, P), dtype=f32),
        "trilm": np.tril(np.ones((P, P), dtype=f32)).T.copy(),
        "negm": _make_negm(),
    }
    qb = be1 @ Wq2
    kb = be1 @ Wk2
    vb = be1 @ Wv2
    b1e = be2 @ W1 + b1
    out["qb"] = np.ascontiguousarray(qb.reshape(CCH, P).T).astype(f32)
    out["kb"] = np.ascontiguousarray(kb.reshape(CCH, P).T).astype(f32)
    out["vb"] = vb[None, :].astype(f32)
    out["bo"] = bo[None, :].astype(f32)
    out["b1c"] = np.ascontiguousarray(b1e.reshape(FCH, P).T).astype(f32)
    out["b2"] = b2[None, :].astype(f32)
    flags = set()
    for name, vec in (("qb", qb), ("kb", kb), ("vb", vb),
                      ("bo", bo), ("b1", b1e), ("b2", b2)):
        if np.any(vec != 0):
            flags.add(name)
    return out, frozenset(flags)


_PROGRAM_CACHE = {}


def _get_program(bl, flags):
    key = (bl, flags)
    if key not in _PROGRAM_CACHE:
        if flags:
            _PROGRAM_CACHE[key] = build_program(
                bl, flags, tr_split=True, ps1_bufs=8)
        else:
            _PROGRAM_CACHE[key] = build_program_v2(bl)
    return _PROGRAM_CACHE[key]


def kernel(x, Wq, Wk, Wv, Wo, bo, W1, b1, W2, b2, g1, be1, g2, be2, **kw):
    from concourse.bass_utils import run_bass_kernel_spmd

    import ml_dtypes

    args = [np.asarray(a, dtype=np.float32) for a in
            (x, Wq, Wk, Wv, Wo, bo, W1, b1, W2, b2, g1, be1, g2, be2)]
    x = args[0]
    _, flags = _probe_flags(*args[1:])
    if flags:
        wmap, flags = prep_weights(*args[1:])
    else:
        wmap = prep_weights_v2(*args[1:])
        x = x.astype(ml_dtypes.bfloat16)
    nc = _get_program(BL, flags)
    xs = x.reshape(NCORES, BL, T, C)
    in_maps = []
    for c in range(NCORES):
        m = {"x": np.ascontiguousarray(xs[c])}
        m.update(wmap)
        in_maps.append(m)
    res = run_bass_kernel_spmd(nc, in_maps, list(range(NCORES)), **kw)
    global _last_results
    _last_results = res
    y = np.stack([res.results[i]["y"] for i in range(NCORES)], axis=0)
    return y.reshape(B, T, C)


def _probe_flags(Wq, Wk, Wv, Wo, bo, W1, b1, W2, b2, g1, be1, g2, be2):
    qb = be1 @ Wq.transpose(1, 0, 2).reshape(C, HD)
    kb = be1 @ Wk.transpose(1, 0, 2).reshape(C, HD)
    vb = be1 @ Wv.transpose(1, 0, 2).reshape(C, HD)
    b1e = be2 @ W1 + b1
    flags = set()
    for name, vec in (("qb", qb), ("kb", kb), ("vb", vb),
                      ("bo", bo), ("b1", b1e), ("b2", b2)):
        if np.any(vec != 0):
            flags.add(name)
    return None, frozenset(flags)


_last_results = None
